# revision 23
# baseline (speedup 1.0000x reference)
"""AssociationLoss Trainium2 kernel (8 NeuronCores, SPMD).

Sharding: data-parallel over N=2 images (cores 0-3 -> image 0, cores 4-7 ->
image 1); within an image the [HW, HW] similarity work is sharded row-wise,
1024 rows per core. The spatially-aggregated x2 (spagg) is computed
column-sharded and all-gathered per image group; per-column argmax/max info is
produced locally per shard and the tiny cross-gather epilogue
(indices = col_argmax[row_argmax], BCE mean) runs on host over 16KB/core.

Precision notes: matmuls run in float32r (full fp32 storage, reduced-precision
PE pass). Row-standardized softmax scores are invariant to per-row scaling, so
the row-side (lhsT) operands are used unnormalized; only column sides are
L2-normalized.
"""
import sys, os
sys.path.insert(0, "/opt/trn_rl_repo")
import numpy as np
from concourse import bass, mybir, tile, bacc
from concourse import bass_utils

F32 = mybir.dt.float32
F32R = mybir.dt.float32r
BF16 = mybir.dt.bfloat16
U32 = mybir.dt.uint32
AF = mybir.ActivationFunctionType
ALU = mybir.AluOpType

C = 512          # channels
L = 4096         # H*W
SH = 1024        # rows per core (L / 4 cores per image)
CCH = 4          # channel chunks of 128
NB = L // 512    # 8 free-dim chunks of 512
EPS = 1e-10
N_CORES = 8

MMDT = F32R      # matmul operand dtype


def build():
    nc = bacc.Bacc("TRN2", target_bir_lowering=False, debug=False,
                   num_devices=N_CORES)
    x2_e = nc.declare_dram_parameter("x2", [C, L], MMDT, isOutput=False)
    x2s_e = nc.declare_dram_parameter("x2s", [C, SH], MMDT, isOutput=False)
    x2t_e = nc.declare_dram_parameter("x2t", [L, C], MMDT, isOutput=False)
    x1_e = nc.declare_dram_parameter("x1", [C, L], MMDT, isOutput=False)
    x1s_e = nc.declare_dram_parameter("x1s", [C, SH], MMDT, isOutput=False)
    gt_e = nc.declare_dram_parameter("gtf", [1, L], BF16, isOutput=False)
    cst_e = nc.declare_dram_parameter("consts", [129, 128], MMDT, isOutput=False)
    out_e = nc.declare_dram_parameter("out", [4, 128, 8], F32, isOutput=True)

    with tile.TileContext(nc) as tc:
        with nc.allow_low_precision(reason="float32r tiles are fp32 storage"), \
             tc.tile_pool(name="sb", bufs=1) as sb, \
             tc.tile_pool(name="ps", bufs=1, space="PSUM") as ps, \
             tc.tile_pool(name="dr", bufs=1, space="DRAM") as dr:
            _build_body(nc, tc, sb, ps, dr,
                        x2_e, x2s_e, x2t_e, x1_e, x1s_e, gt_e, cst_e, out_e)
    nc.compile()
    return nc


def _norm_full_streamed(nc, sb, ps, src_e, out_tiles, ones128, pref):
    """Column-normalize a [C, L] DRAM tensor into 4x[128, L] resident tiles,
    streaming the source twice in [128, 512] chunks."""
    npt = ps.tile([128, L], F32, tag="p1", name=f"{pref}npt")
    nps = [npt[:, n*512:(n+1)*512] for n in range(NB)]
    for cc in range(CCH):
        for n in range(NB):
            ld = sb.tile([128, 512], MMDT, tag="bigload", bufs=2,
                         name=f"{pref}ld{cc}_{n}")
            nc.sync.dma_start(out=ld[:], in_=src_e[cc*128:(cc+1)*128,
                                                   n*512:(n+1)*512])
            sq = sb.tile([128, 512], MMDT, tag="sqchunk", bufs=2,
                         name=f"{pref}sq{cc}_{n}")
            nc.scalar.activation(sq[:], ld[:], AF.Square)
            nc.tensor.matmul(nps[n], lhsT=ones128[:], rhs=sq[:],
                             start=(cc == 0), stop=(cc == CCH - 1))
    for n in range(NB):
        sq_n = sb.tile([128, 512], F32, tag="nsq", bufs=2, name=f"{pref}nsq{n}")
        nc.scalar.activation(sq_n[:], nps[n], AF.Sqrt)
        rn = sb.tile([128, 512], MMDT, tag="rnorm", bufs=2, name=f"{pref}rn{n}")
        nc.vector.reciprocal(rn[:], sq_n[:])
        for cc in range(CCH):
            ld = sb.tile([128, 512], MMDT, tag="bigload", bufs=2,
                         name=f"{pref}ld2{cc}_{n}")
            nc.sync.dma_start(out=ld[:], in_=src_e[cc*128:(cc+1)*128,
                                                   n*512:(n+1)*512])
            nc.vector.tensor_tensor(out_tiles[cc][:, n*512:(n+1)*512],
                                    ld[:], rn[:], ALU.mult)


def _row_stats(nc, sb, p1, tag_pref, ib, with_index, opack_val, opack_idx=None,
               gt_bc=None, opack_gtm=None):
    """p1 = [128, L] PSUM of similarity rows -> standardized-softmax stats.
    Writes softmax max value (+ argmax / value-matched gt gather) to opack."""
    s12 = sb.tile([128, L], F32, tag="srow", bufs=1, name=f"{tag_pref}s{ib}")
    rsum = sb.tile([128, 1], F32, tag="t_rsum", bufs=4, name=f"{tag_pref}rsum{ib}")
    nc.scalar.activation(s12[:], p1[:], AF.Copy, accum_out=rsum[:])
    trash1 = sb.tile([128, L], BF16, tag="trash", bufs=2, name=f"{tag_pref}t1{ib}")
    rsq = sb.tile([128, 1], F32, tag="t_rsq", bufs=4, name=f"{tag_pref}rsq{ib}")
    nc.scalar.activation(trash1[:], s12[:], AF.Square, accum_out=rsq[:])
    m8 = sb.tile([128, 8], F32, tag="m8", bufs=2, name=f"{tag_pref}m8{ib}")
    nc.vector.max(m8[:], s12[:])
    if with_index:
        mi8 = sb.tile([128, 8], U32, tag="mi8", bufs=2, name=f"{tag_pref}mi8{ib}")
        nc.vector.max_index(mi8[:], m8[:], s12[:])
        nc.vector.tensor_copy(opack_idx[:, ib:ib+1], mi8[:, 0:1])

    def tiny(nm):
        return sb.tile([128, 1], F32, tag="tiny", bufs=24, name=f"{tag_pref}{nm}{ib}")
    mean = tiny("mean")
    nc.vector.tensor_scalar_mul(mean[:], rsum[:], 1.0 / L)
    t1 = tiny("t1")
    nc.vector.tensor_tensor(t1[:], rsum[:], mean[:], ALU.mult)
    t2 = tiny("t2")
    nc.vector.tensor_tensor(t2[:], rsq[:], t1[:], ALU.subtract)
    std = tiny("std")
    nc.scalar.activation(std[:], t2[:], AF.Sqrt, scale=1.0 / (L - 1))
    stde = tiny("stde")
    nc.vector.tensor_scalar_add(stde[:], std[:], EPS)
    rinv = tiny("rinv")
    nc.vector.reciprocal(rinv[:], stde[:])
    mr = tiny("mr")
    nc.vector.tensor_tensor(mr[:], mean[:], rinv[:], ALU.mult)
    nb_ = tiny("nb")
    nc.vector.tensor_scalar_mul(nb_[:], mr[:], -1.0)
    rse = tiny("rse")
    trash2 = sb.tile([128, L], BF16, tag="trash", bufs=2, name=f"{tag_pref}t2{ib}")
    nc.scalar.activation(trash2[:], s12[:], AF.Exp, scale=rinv[:], bias=nb_[:],
                         accum_out=rse[:])
    t3 = tiny("t3")
    nc.vector.tensor_tensor(t3[:], m8[:, 0:1], rinv[:], ALU.mult)
    t4 = tiny("t4")
    nc.vector.tensor_tensor(t4[:], t3[:], nb_[:], ALU.add)
    em = tiny("em")
    nc.scalar.activation(em[:], t4[:], AF.Exp)
    rr2 = tiny("rr2")
    nc.vector.reciprocal(rr2[:], rse[:])
    nc.vector.tensor_tensor(opack_val[:, ib:ib+1], em[:], rr2[:], ALU.mult)
    if gt_bc is not None:
        # gtm[j] = gt[argmax_i s21[j, i]] via exact value-match one-hot
        oh = sb.tile([128, L], BF16, tag="trash", bufs=2, name=f"{tag_pref}oh{ib}")
        nc.gpsimd.tensor_scalar(oh[:], s12[:], m8[:, 0:1], None, ALU.is_equal)
        prod = sb.tile([128, L], BF16, tag="trash", bufs=2, name=f"{tag_pref}pr{ib}")
        nc.gpsimd.tensor_tensor(prod[:], oh[:], gt_bc[:], ALU.mult)
        nc.vector.tensor_reduce(opack_gtm[:, ib:ib+1], prod[:],
                                axis=mybir.AxisListType.X, op=ALU.add)


def _build_body(nc, tc, sb, ps, dr, x2_e, x2s_e, x2t_e, x1_e, x1s_e, gt_e, cst_e,
                out_e):
    # ---- constants (host-provided: row 0 = ones, rows 1..128 = identity) ----
    ones1 = sb.tile([1, 128], MMDT, name="ones1")
    nc.sync.dma_start(out=ones1[:], in_=cst_e[0:1, :])
    ones128 = sb.tile([128, 128], MMDT, name="ones128")
    nc.gpsimd.partition_broadcast(ones128[:], ones1[:])
    id128 = sb.tile([128, 128], MMDT, name="id128")
    nc.sync.dma_start(out=id128[:], in_=cst_e[1:129, :])
    gt_row = sb.tile([1, L], BF16, name="gt_row")
    nc.sync.dma_start(out=gt_row[:], in_=gt_e[:, :])
    gt_bc = sb.tile([128, L], BF16, name="gt_bc")
    nc.gpsimd.partition_broadcast(gt_bc[:], gt_row[:])

    opack = [sb.tile([128, 8], F32, tag=f"op{k}", name=f"opack{k}") for k in range(4)]

    # ---- phase A: normalize x2 columns into resident x2n (tag bigB) ----
    x2sh = []
    for cc in range(CCH):
        t = sb.tile([128, SH], MMDT, tag=f"shA{cc}", bufs=1, name=f"x2sh{cc}")
        nc.sync.dma_start(out=t[:], in_=x2s_e[cc*128:(cc+1)*128, :])
        x2sh.append(t)
    x2n = [sb.tile([128, L], MMDT, tag=f"bigB{cc}", bufs=1, name=f"x2n{cc}")
           for cc in range(CCH)]
    _norm_full_streamed(nc, sb, ps, x2_e, x2n, ones128, "A")

    # ---- phase B: sim_self row stats (rows of shard; raw lhsT is fine) ----
    spack = sb.tile([128, 16], MMDT, name="spack")
    for ib in range(8):
        p1 = ps.tile([128, L], F32, tag="p1", name=f"B_p1_{ib}")
        for n in range(NB):
            for cc in range(CCH):
                nc.tensor.matmul(p1[:, n*512:(n+1)*512],
                                 lhsT=x2sh[cc][:, ib*128:(ib+1)*128],
                                 rhs=x2n[cc][:, n*512:(n+1)*512],
                                 start=(cc == 0), stop=(cc == CCH - 1))
        bno = sb.tile([128, 6 * NB], F32, tag="bno", bufs=2, name=f"bno{ib}")
        for n in range(NB):
            nc.vector.bn_stats(bno[:, n*6:(n+1)*6], p1[:, n*512:(n+1)*512])
        bna = sb.tile([128, 2], F32, tag="bna", bufs=2, name=f"bna{ib}")
        nc.vector.bn_aggr(bna[:], bno[:])

        def tiny(nm):
            return sb.tile([128, 1], F32, tag="tiny", bufs=24, name=f"B{nm}{ib}")
        std = tiny("std")
        nc.scalar.activation(std[:], bna[:, 1:2], AF.Sqrt, scale=L / (L - 1.0))
        stde = tiny("stde")
        nc.vector.tensor_scalar_add(stde[:], std[:], EPS)
        rinv = tiny("rinv")
        nc.vector.reciprocal(rinv[:], stde[:])
        nc.vector.tensor_copy(spack[:, ib:ib+1], rinv[:])
        mr = tiny("mr")
        nc.vector.tensor_tensor(mr[:], bna[:, 0:1], rinv[:], ALU.mult)
        nc.vector.tensor_scalar_mul(spack[:, 8+ib:9+ib], mr[:], -1.0)
    stT = ps.tile([16, 128], MMDT, tag="p1", name="stT")
    nc.tensor.transpose(stT[:], spack[:], id128[:])
    st = sb.tile([16, 128], MMDT, name="st")
    nc.vector.tensor_copy(st[:], stT[:])
    # bias rows to partition 0 as one [1, 1024] row (DRAM bounce)
    b_dram = dr.tile([1, SH], MMDT, name="b_dram")
    nc.sync.dma_start(out=b_dram[:, :].rearrange("o (a b) -> (o a) b", a=8),
                      in_=st[8:16, :])
    b_row = sb.tile([1, SH], MMDT, name="b_row")
    nc.sync.dma_start(out=b_row[:], in_=b_dram[:, :])
    r_dram = dr.tile([1, SH], MMDT, name="r_dram")
    nc.sync.dma_start(out=r_dram[:, :].rearrange("o (a b) -> (o a) b", a=8),
                      in_=st[0:8, :])
    r_row = sb.tile([1, SH], MMDT, name="r_row")
    nc.sync.dma_start(out=r_row[:], in_=r_dram[:, :])

    # ---- phase C: sim_self exp + agg + spagg combine + renormalize ----
    cc_in = dr.tile([C, SH], MMDT, name="cc_in")
    xnn = {}
    for ic in range(2):
        rbc = sb.tile([128, 512], MMDT, tag="rbc", bufs=1, name=f"rbc{ic}")
        nc.gpsimd.partition_broadcast(rbc[:], r_row[0:1, ic*512:(ic+1)*512])
        ys = []
        for cc in range(CCH):
            y = sb.tile([128, 512], MMDT, tag=f"y{cc}", bufs=1, name=f"y{cc}_{ic}")
            nc.vector.tensor_tensor(y[:], x2sh[cc][:, ic*512:(ic+1)*512], rbc[:],
                                    ALU.mult)
            ys.append(y)
        cps = ps.tile([128, L], F32, tag="p1", name=f"cps{ic}")
        agg_ps = [cps[:, 1024+cc*512:1024+(cc+1)*512] for cc in range(CCH)]
        rse_ps = cps[0:1, 3072:3584]
        for jb in range(32):
            zt = cps[:, (jb % 2)*512:(jb % 2 + 1)*512]
            for cc in range(CCH):
                nc.tensor.matmul(zt, lhsT=x2n[cc][:, jb*128:(jb+1)*128],
                                 rhs=ys[cc][:], start=(cc == 0), stop=False)
            nc.tensor.matmul(zt, lhsT=ones1[:],
                             rhs=b_row[0:1, ic*512:(ic+1)*512],
                             start=False, stop=True)
            ez = sb.tile([128, 512], MMDT, tag="ez", bufs=2, name=f"ez{ic}_{jb}")
            nc.scalar.activation(ez[:], zt, AF.Exp)
            xt = sb.tile([128, C], MMDT, tag="xt", bufs=2, name=f"xt{ic}_{jb}")
            nc.sync.dma_start(out=xt[:], in_=x2t_e[jb*128:(jb+1)*128, :])
            for cc in range(CCH):
                nc.tensor.matmul(agg_ps[cc], lhsT=xt[:, cc*128:(cc+1)*128],
                                 rhs=ez[:], start=(jb == 0), stop=(jb == 31))
            nc.tensor.matmul(rse_ps, lhsT=ones128[:, 0:1], rhs=ez[:],
                             start=(jb == 0), stop=(jb == 31))
        rrse = sb.tile([1, 512], F32, tag="rrse", bufs=1, name=f"rrse{ic}")
        nc.vector.reciprocal(rrse[:], rse_ps)
        rrb = sb.tile([128, 512], F32, tag="rrb", bufs=1, name=f"rrb{ic}")
        nc.gpsimd.partition_broadcast(rrb[:], rrse[:])
        nps2 = cps[:, 3584:4096]
        xss_l = []
        for cc in range(CCH):
            tmp = sb.tile([128, 512], F32, tag="tmpc", bufs=1, name=f"tmp{cc}_{ic}")
            nc.vector.tensor_tensor(tmp[:], agg_ps[cc], rrb[:], ALU.mult)
            xss = sb.tile([128, 512], MMDT, tag="xss", bufs=4, name=f"xss{cc}_{ic}")
            nc.vector.tensor_tensor(xss[:], tmp[:], x2sh[cc][:, ic*512:(ic+1)*512],
                                    ALU.add)
            xss_l.append(xss)
            sq2 = sb.tile([128, 512], MMDT, tag="sqchunk", bufs=2,
                          name=f"sq2{cc}_{ic}")
            nc.scalar.activation(sq2[:], xss[:], AF.Square)
            nc.tensor.matmul(nps2, lhsT=ones128[:], rhs=sq2[:],
                             start=(cc == 0), stop=(cc == CCH - 1))
        sq_n2 = sb.tile([128, 512], F32, tag="nsq", bufs=2, name=f"rnb2sq{ic}")
        nc.scalar.activation(sq_n2[:], nps2, AF.Sqrt)
        rnb2 = sb.tile([128, 512], MMDT, tag="rnb2", bufs=1, name=f"rnb2{ic}")
        nc.vector.reciprocal(rnb2[:], sq_n2[:])
        for cc in range(CCH):
            xo = sb.tile([128, 512], MMDT, tag="xnn", bufs=8, name=f"xnn{cc}_{ic}")
            nc.vector.tensor_tensor(xo[:], xss_l[cc][:], rnb2[:], ALU.mult)
            xnn[(cc, ic)] = xo
            nc.sync.dma_start(out=cc_in[cc*128:(cc+1)*128, ic*512:(ic+1)*512],
                              in_=xo[:])

    # ---- all-gather normalized x2_new within each image group ----
    cc_out = dr.tile([4, C, SH], MMDT, name="cc_out")
    nc.gpsimd.collective_compute(
        "AllGather", ALU.bypass,
        replica_groups=[[0, 1, 2, 3], [4, 5, 6, 7]],
        ins=[cc_in[:].opt()], outs=[cc_out[:].opt()])

    # ---- phase D: normalize x1 columns into resident x1n (tag bigB) ----
    x1n = [sb.tile([128, L], MMDT, tag=f"bigB{cc}", bufs=1, name=f"x1n{cc}")
           for cc in range(CCH)]
    _norm_full_streamed(nc, sb, ps, x1_e, x1n, ones128, "D")
    x1sh = []
    for cc in range(CCH):
        t = sb.tile([128, SH], MMDT, tag=f"shA{cc}", bufs=1, name=f"x1sh{cc}")
        nc.sync.dma_start(out=t[:], in_=x1s_e[cc*128:(cc+1)*128, :])
        x1sh.append(t)

    # ---- phase F: raw21 columns (fully local; overlaps the all-gather) ----
    for jb in range(8):
        p1 = ps.tile([128, L], F32, tag="p1", name=f"F_p1_{jb}")
        for n in range(NB):
            for cc in range(CCH):
                nc.tensor.matmul(p1[:, n*512:(n+1)*512],
                                 lhsT=xnn[(cc, jb // 4)][:, (jb % 4)*128:(jb % 4 + 1)*128],
                                 rhs=x1n[cc][:, n*512:(n+1)*512],
                                 start=(cc == 0), stop=(cc == CCH - 1))
        _row_stats(nc, sb, p1, "F", jb, with_index=False, opack_val=opack[3],
                   gt_bc=gt_bc, opack_gtm=opack[2])

    # ---- read back gathered x2_new into tag bigB (after x1n's last use) ----
    xg = []
    for cc in range(CCH):
        t = sb.tile([128, L], MMDT, tag=f"bigB{cc}", bufs=1, name=f"xg{cc}")
        for rr in range(4):
            nc.sync.dma_start(out=t[:, rr*SH:(rr+1)*SH],
                              in_=cc_out[rr, cc*128:(cc+1)*128, :])
        xg.append(t)

    # ---- phase E: raw12 rows ----
    for ib in range(8):
        p1 = ps.tile([128, L], F32, tag="p1", name=f"E_p1_{ib}")
        for n in range(NB):
            for cc in range(CCH):
                nc.tensor.matmul(p1[:, n*512:(n+1)*512],
                                 lhsT=x1sh[cc][:, ib*128:(ib+1)*128],
                                 rhs=xg[cc][:, n*512:(n+1)*512],
                                 start=(cc == 0), stop=(cc == CCH - 1))
        _row_stats(nc, sb, p1, "E", ib, with_index=True, opack_val=opack[1],
                   opack_idx=opack[0])

    # ---- outputs ----
    for k in range(4):
        nc.sync.dma_start(out=out_e[k, :, :], in_=opack[k][:])


_NC_CACHE = None


def _get_nc():
    global _NC_CACHE
    if _NC_CACHE is None:
        _NC_CACHE = build()
    return _NC_CACHE


def _consts():
    c = np.zeros((129, 128), np.float32)
    c[0, :] = 1.0
    c[1:129, :] = np.eye(128, dtype=np.float32)
    return c


def _unpack(v):
    # v: [128, 8] packed [p, b] -> flat vec[b*128 + p]
    return v.T.reshape(-1)


def kernel(x1, x2, gt1, _want_profile=False):
    N = x1.shape[0]
    assert N == 2 and x1.shape == (2, 512, 64, 64)
    x1m = x1.reshape(N, C, L).astype(np.float32)
    x2m = x2.reshape(N, C, L).astype(np.float32)
    gtf = gt1.reshape(N, L).astype(np.float32)

    in_maps = []
    for core in range(N_CORES):
        g, r = core // 4, core % 4
        sl = slice(r * SH, (r + 1) * SH)
        in_maps.append({
            "x2": np.ascontiguousarray(x2m[g]),
            "x2s": np.ascontiguousarray(x2m[g][:, sl]),
            "x2t": np.ascontiguousarray(x2m[g].T),
            "x1": np.ascontiguousarray(x1m[g]),
            "x1s": np.ascontiguousarray(x1m[g][:, sl]),
            "gtf": np.ascontiguousarray(gtf[g:g+1]).astype(mybir.dt.np(BF16)),
            "consts": _consts(),
        })
    nc = _get_nc()
    res = bass_utils.run_bass_kernel_spmd(
        nc, in_maps, core_ids=list(range(N_CORES)),
        trace=_want_profile)

    total = 0.0
    cnt = 0.0
    for g in range(N):
        packs = [res.results[g * 4 + r]["out"] for r in range(4)]
        mid = np.concatenate([_unpack(p[0]) for p in packs])
        assoc = np.concatenate([_unpack(p[1]) for p in packs])
        gtm = np.concatenate([_unpack(p[2]) for p in packs])
        msim = np.concatenate([_unpack(p[3]) for p in packs])
        mi = mid.astype(np.int64)
        reassoc = msim[mi]
        gta = gtm[mi]
        gtimg = gtf[g]
        sel = (gtimg == gta) & (gt1.reshape(N, L)[g] != 255)
        sim = assoc.astype(np.float64) * reassoc.astype(np.float64)
        with np.errstate(divide="ignore"):
            term = np.minimum(-np.log(sim), 100.0)
        total += float((term * sel).sum())
        cnt += float(sel.sum())
    loss = total / max(cnt, 1.0) if cnt > 0 else 0.0
    out = np.float32(loss)
    if _want_profile:
        return out, res
    return out


# revision 24
# speedup vs baseline: 1.4052x; 1.4052x over previous
"""AssociationLoss Trainium2 kernel (8 NeuronCores, SPMD).

Sharding: data-parallel over N=2 images (cores 0-3 -> image 0, cores 4-7 ->
image 1); within an image the [HW, HW] similarity work is sharded row-wise,
1024 rows per core. The spatially-aggregated x2 (spagg) is computed
column-sharded and all-gathered per image group; per-row/column argmax + max
softmax values are produced per shard and the tiny index-chase epilogue
(indices = col_argmax[row_argmax], gt compare, BCE mean over ~16KB/core)
runs on host.

Precision: matmuls run in float32r (fp32 storage, single-pass reduced-precision
PE). Row-standardized softmax scores are invariant to per-row scaling, so
row-side (lhsT) operands are used unnormalized; only column sides are
L2-normalized. 1/sqrt is computed as exp(-0.5*ln(x)) so the whole kernel uses
a single ACT table set (natural_log_exp).
"""
import sys, os
sys.path.insert(0, "/opt/trn_rl_repo")
import numpy as np
from concourse import bass, mybir, tile, bacc
from concourse import bass_utils

F32 = mybir.dt.float32
F32R = mybir.dt.float32r
BF16 = mybir.dt.bfloat16
U32 = mybir.dt.uint32
AF = mybir.ActivationFunctionType
ALU = mybir.AluOpType

C = 512
L = 4096
SH = 1024
CCH = 4
NB = L // 512
EPS = 1e-10
N_CORES = 8

MMDT = F32R


def build():
    nc = bacc.Bacc("TRN2", target_bir_lowering=False, debug=False,
                   num_devices=N_CORES)
    x2_e = nc.declare_dram_parameter("x2", [C, L], MMDT, isOutput=False)
    x2s_e = nc.declare_dram_parameter("x2s", [C, SH], MMDT, isOutput=False)
    x2t_e = nc.declare_dram_parameter("x2t", [L, C], MMDT, isOutput=False)
    x1_e = nc.declare_dram_parameter("x1", [C, L], MMDT, isOutput=False)
    x1s_e = nc.declare_dram_parameter("x1s", [C, SH], MMDT, isOutput=False)
    cst_e = nc.declare_dram_parameter("consts", [129, 128], MMDT, isOutput=False)
    out_e = nc.declare_dram_parameter("out", [4, 128, 8], F32, isOutput=True)

    with tile.TileContext(nc) as tc:
        with nc.allow_low_precision(reason="float32r tiles are fp32 storage"), \
             tc.tile_pool(name="sb", bufs=1) as sb, \
             tc.tile_pool(name="ps", bufs=1, space="PSUM") as ps, \
             tc.tile_pool(name="dr", bufs=1, space="DRAM") as dr:
            _build_body(nc, tc, sb, ps, dr,
                        x2_e, x2s_e, x2t_e, x1_e, x1s_e, cst_e, out_e)
    nc.compile()
    return nc


def _rsqrt_act(nc, sb, dst, src_ap, scale, pref, shape=None, dtype=None):
    """dst = (scale*src)^-0.5 via exp(-0.5*ln(scale*src)) — stays in the
    natural_log_exp ACT table set."""
    shape = shape or [128, 512]
    lnv = sb.tile(shape, F32, tag="lnscr", bufs=2, name=f"{pref}ln")
    nc.scalar.activation(lnv[:], src_ap, AF.Ln, scale=scale)
    nc.scalar.activation(dst, lnv[:], AF.Exp, scale=-0.5)


def _norm_full_streamed(nc, sb, ps, src_e, out_tiles, ones128, pref):
    """Column-normalize a [C, L] DRAM tensor into 4x[128, L] resident tiles,
    streaming the source twice in [128, 512] chunks."""
    npa = ps.tile([128, L // 2], F32, tag="p1a", name=f"{pref}npa")
    npb = ps.tile([128, L // 2], F32, tag="p1b", name=f"{pref}npb")
    nps = [npa[:, n*512:(n+1)*512] for n in range(4)] + \
          [npb[:, n*512:(n+1)*512] for n in range(4)]
    for cc in range(CCH):
        for n in range(NB):
            ld = sb.tile([128, 512], MMDT, tag="bigload", bufs=2,
                         name=f"{pref}ld{cc}_{n}")
            nc.sync.dma_start(out=ld[:], in_=src_e[cc*128:(cc+1)*128,
                                                   n*512:(n+1)*512])
            sq = sb.tile([128, 512], MMDT, tag="sqchunk", bufs=2,
                         name=f"{pref}sq{cc}_{n}")
            nc.scalar.activation(sq[:], ld[:], AF.Square)
            nc.tensor.matmul(nps[n], lhsT=ones128[:], rhs=sq[:],
                             start=(cc == 0), stop=(cc == CCH - 1))
    for n in range(NB):
        rn = sb.tile([128, 512], MMDT, tag="rnorm", bufs=2, name=f"{pref}rn{n}")
        _rsqrt_act(nc, sb, rn[:], nps[n], 1.0, f"{pref}rs{n}")
        for cc in range(CCH):
            ld = sb.tile([128, 512], MMDT, tag="bigload", bufs=2,
                         name=f"{pref}ld2{cc}_{n}")
            nc.sync.dma_start(out=ld[:], in_=src_e[cc*128:(cc+1)*128,
                                                   n*512:(n+1)*512])
            nc.vector.tensor_tensor(out_tiles[cc][:, n*512:(n+1)*512],
                                    ld[:], rn[:], ALU.mult)


def _row_stats(nc, sb, p1a, p1b, tag_pref, ib, opack_val, opack_idx):
    """p1a/p1b = [128, 2048] PSUM halves of similarity rows. Standardized
    softmax stats per row; writes argmax and max softmax value to opack."""
    s12 = sb.tile([128, L], F32, tag="srow", bufs=2, name=f"{tag_pref}s{ib}")
    nc.scalar.activation(s12[:, 0:L//2], p1a[:], AF.Copy)
    nc.scalar.activation(s12[:, L//2:L], p1b[:], AF.Copy)
    bno = sb.tile([128, 6 * NB], F32, tag="bno", bufs=2, name=f"{tag_pref}bno{ib}")
    for n in range(4):
        nc.vector.bn_stats(bno[:, n*6:(n+1)*6], p1a[:, n*512:(n+1)*512])
    for n in range(4):
        nc.vector.bn_stats(bno[:, 24+n*6:24+(n+1)*6], p1b[:, n*512:(n+1)*512])
    bna = sb.tile([128, 2], F32, tag="bna", bufs=2, name=f"{tag_pref}bna{ib}")
    nc.vector.bn_aggr(bna[:], bno[:])
    m8 = sb.tile([128, 8], F32, tag="m8", bufs=2, name=f"{tag_pref}m8{ib}")
    nc.vector.max(m8[:], s12[:])
    mi8 = sb.tile([128, 8], U32, tag="mi8", bufs=2, name=f"{tag_pref}mi8{ib}")
    nc.vector.max_index(mi8[:], m8[:], s12[:])
    nc.vector.tensor_copy(opack_idx[:, ib:ib+1], mi8[:, 0:1])

    def tiny(nm):
        return sb.tile([128, 1], F32, tag="tiny", bufs=16, name=f"{tag_pref}{nm}{ib}")
    # rinv = 1/std (unbiased) = exp(-0.5*ln(var*L/(L-1)))
    rinv = tiny("rinv")
    _rsqrt_act(nc, sb, rinv[:], bna[:, 1:2], L / (L - 1.0), f"{tag_pref}ri{ib}",
               shape=[128, 1])
    mr = tiny("mr")
    nc.vector.tensor_tensor(mr[:], bna[:, 0:1], rinv[:], ALU.mult)
    nb_ = tiny("nb")
    nc.vector.tensor_scalar_mul(nb_[:], mr[:], -1.0)
    rse = tiny("rse")
    trash = sb.tile([128, L], BF16, tag="trash", bufs=2, name=f"{tag_pref}t2{ib}")
    nc.scalar.activation(trash[:], s12[:], AF.Exp, scale=rinv[:], bias=nb_[:],
                         accum_out=rse[:])
    t3 = tiny("t3")
    nc.vector.tensor_tensor(t3[:], m8[:, 0:1], rinv[:], ALU.mult)
    t4 = tiny("t4")
    nc.vector.tensor_tensor(t4[:], t3[:], nb_[:], ALU.add)
    em = tiny("em")
    nc.scalar.activation(em[:], t4[:], AF.Exp)
    rr2 = tiny("rr2")
    nc.vector.reciprocal(rr2[:], rse[:])
    nc.vector.tensor_tensor(opack_val[:, ib:ib+1], em[:], rr2[:], ALU.mult)


def _build_body(nc, tc, sb, ps, dr, x2_e, x2s_e, x2t_e, x1_e, x1s_e, cst_e,
                out_e):
    # ---- constants (host-provided: row 0 = ones, rows 1..128 = identity) ----
    ones1 = sb.tile([1, 128], MMDT, name="ones1")
    nc.sync.dma_start(out=ones1[:], in_=cst_e[0:1, :])
    ones128 = sb.tile([128, 128], MMDT, name="ones128")
    nc.gpsimd.partition_broadcast(ones128[:], ones1[:])
    id128 = sb.tile([128, 128], MMDT, name="id128")
    nc.sync.dma_start(out=id128[:], in_=cst_e[1:129, :])

    opack = [sb.tile([128, 8], F32, tag=f"op{k}", name=f"opack{k}") for k in range(4)]

    # ---- phase A: normalize x2 columns into resident x2n ----
    x2sh = []
    for cc in range(CCH):
        t = sb.tile([128, SH], MMDT, tag=f"shA{cc}", bufs=1, name=f"x2sh{cc}")
        nc.sync.dma_start(out=t[:], in_=x2s_e[cc*128:(cc+1)*128, :])
        x2sh.append(t)
    x2n = [sb.tile([128, L], MMDT, tag=f"bigB{cc}", bufs=1, name=f"x2n{cc}")
           for cc in range(CCH)]
    _norm_full_streamed(nc, sb, ps, x2_e, x2n, ones128, "A")

    # ---- phase B: sim_self row stats (raw lhsT; row-scale invariant) ----
    spack = sb.tile([128, 16], MMDT, name="spack")
    for ib in range(8):
        pa = ps.tile([128, L // 2], F32, tag="p1a", name=f"B_pa{ib}")
        pb = ps.tile([128, L // 2], F32, tag="p1b", name=f"B_pb{ib}")
        for n in range(NB):
            dst = pa[:, n*512:(n+1)*512] if n < 4 else pb[:, (n-4)*512:(n-3)*512]
            for cc in range(CCH):
                nc.tensor.matmul(dst,
                                 lhsT=x2sh[cc][:, ib*128:(ib+1)*128],
                                 rhs=x2n[cc][:, n*512:(n+1)*512],
                                 start=(cc == 0), stop=(cc == CCH - 1))
        bno = sb.tile([128, 6 * NB], F32, tag="bno", bufs=2, name=f"Bbno{ib}")
        for n in range(4):
            nc.vector.bn_stats(bno[:, n*6:(n+1)*6], pa[:, n*512:(n+1)*512])
        for n in range(4):
            nc.vector.bn_stats(bno[:, 24+n*6:24+(n+1)*6], pb[:, n*512:(n+1)*512])
        bna = sb.tile([128, 2], F32, tag="bna", bufs=2, name=f"Bbna{ib}")
        nc.vector.bn_aggr(bna[:], bno[:])
        rinv = sb.tile([128, 1], F32, tag="tiny", bufs=16, name=f"Brinv{ib}")
        _rsqrt_act(nc, sb, rinv[:], bna[:, 1:2], L / (L - 1.0), f"Bri{ib}",
                   shape=[128, 1])
        nc.vector.tensor_copy(spack[:, ib:ib+1], rinv[:])
        mr = sb.tile([128, 1], F32, tag="tiny", bufs=16, name=f"Bmr{ib}")
        nc.vector.tensor_tensor(mr[:], bna[:, 0:1], rinv[:], ALU.mult)
        nc.vector.tensor_scalar_mul(spack[:, 8+ib:9+ib], mr[:], -1.0)
    stT = ps.tile([16, 128], MMDT, tag="p1a", name="stT")
    nc.tensor.transpose(stT[:], spack[:], id128[:])
    st = sb.tile([16, 128], MMDT, name="st")
    nc.vector.tensor_copy(st[:], stT[:])
    # move rinv/bias rows to partition 0 as [1, 1024] rows (DRAM bounce)
    b_dram = dr.tile([1, SH], MMDT, name="b_dram")
    nc.sync.dma_start(out=b_dram[:, :].rearrange("o (a b) -> (o a) b", a=8),
                      in_=st[8:16, :])
    b_row = sb.tile([1, SH], MMDT, name="b_row")
    nc.sync.dma_start(out=b_row[:], in_=b_dram[:, :])
    r_dram = dr.tile([1, SH], MMDT, name="r_dram")
    nc.sync.dma_start(out=r_dram[:, :].rearrange("o (a b) -> (o a) b", a=8),
                      in_=st[0:8, :])
    r_row = sb.tile([1, SH], MMDT, name="r_row")
    nc.sync.dma_start(out=r_row[:], in_=r_dram[:, :])

    # ---- phase C: sim_self exp + agg + spagg combine + renormalize ----
    cc_in = dr.tile([C, SH], MMDT, name="cc_in")
    xnn = {}
    for ic in range(2):
        rbc = sb.tile([128, 512], MMDT, tag="rbc", bufs=1, name=f"rbc{ic}")
        nc.gpsimd.partition_broadcast(rbc[:], r_row[0:1, ic*512:(ic+1)*512])
        ys = []
        for cc in range(CCH):
            y = sb.tile([128, 512], MMDT, tag=f"y{cc}", bufs=1, name=f"y{cc}_{ic}")
            nc.vector.tensor_tensor(y[:], x2sh[cc][:, ic*512:(ic+1)*512], rbc[:],
                                    ALU.mult)
            ys.append(y)
        pa = ps.tile([128, L // 2], F32, tag="p1a", name=f"C_pa{ic}")
        pb = ps.tile([128, L // 2], F32, tag="p1b", name=f"C_pb{ic}")
        # bank map: pa = [zt0, zt1, agg0, agg1]; pb = [agg2, agg3, rse, np2]
        agg_ps = [pa[:, 1024:1536], pa[:, 1536:2048], pb[:, 0:512], pb[:, 512:1024]]
        rse_ps = pb[0:1, 1024:1536]
        for jb in range(32):
            zt = pa[:, (jb % 2)*512:(jb % 2 + 1)*512]
            for cc in range(CCH):
                nc.tensor.matmul(zt, lhsT=x2n[cc][:, jb*128:(jb+1)*128],
                                 rhs=ys[cc][:], start=(cc == 0), stop=False)
            nc.tensor.matmul(zt, lhsT=ones1[:],
                             rhs=b_row[0:1, ic*512:(ic+1)*512],
                             start=False, stop=True)
            ez = sb.tile([128, 512], MMDT, tag="ez", bufs=2, name=f"ez{ic}_{jb}")
            nc.scalar.activation(ez[:], zt, AF.Exp)
            xt = sb.tile([128, C], MMDT, tag="xt", bufs=2, name=f"xt{ic}_{jb}")
            nc.sync.dma_start(out=xt[:], in_=x2t_e[jb*128:(jb+1)*128, :])
            for cc in range(CCH):
                nc.tensor.matmul(agg_ps[cc], lhsT=xt[:, cc*128:(cc+1)*128],
                                 rhs=ez[:], start=(jb == 0), stop=(jb == 31))
            nc.tensor.matmul(rse_ps, lhsT=ones128[:, 0:1], rhs=ez[:],
                             start=(jb == 0), stop=(jb == 31))
        rrse = sb.tile([1, 512], F32, tag="rrse", bufs=1, name=f"rrse{ic}")
        nc.vector.reciprocal(rrse[:], rse_ps)
        rrb = sb.tile([128, 512], F32, tag="rrb", bufs=1, name=f"rrb{ic}")
        nc.gpsimd.partition_broadcast(rrb[:], rrse[:])
        nps2 = pb[:, 1536:2048]
        xss_l = []
        for cc in range(CCH):
            tmp = sb.tile([128, 512], F32, tag="tmpc", bufs=1, name=f"tmp{cc}_{ic}")
            nc.vector.tensor_tensor(tmp[:], agg_ps[cc], rrb[:], ALU.mult)
            xss = sb.tile([128, 512], MMDT, tag="xss", bufs=4, name=f"xss{cc}_{ic}")
            nc.vector.tensor_tensor(xss[:], tmp[:], x2sh[cc][:, ic*512:(ic+1)*512],
                                    ALU.add)
            xss_l.append(xss)
            sq2 = sb.tile([128, 512], MMDT, tag="sqchunk", bufs=2,
                          name=f"sq2{cc}_{ic}")
            nc.scalar.activation(sq2[:], xss[:], AF.Square)
            nc.tensor.matmul(nps2, lhsT=ones128[:], rhs=sq2[:],
                             start=(cc == 0), stop=(cc == CCH - 1))
        rnb2 = sb.tile([128, 512], MMDT, tag="rnb2", bufs=1, name=f"rnb2{ic}")
        _rsqrt_act(nc, sb, rnb2[:], nps2, 1.0, f"rnb2s{ic}")
        for cc in range(CCH):
            xo = sb.tile([128, 512], MMDT, tag="xnn", bufs=8, name=f"xnn{cc}_{ic}")
            nc.vector.tensor_tensor(xo[:], xss_l[cc][:], rnb2[:], ALU.mult)
            xnn[(cc, ic)] = xo
            nc.sync.dma_start(out=cc_in[cc*128:(cc+1)*128, ic*512:(ic+1)*512],
                              in_=xo[:])

    # ---- all-gather normalized x2_new within each image group ----
    cc_out = dr.tile([4, C, SH], MMDT, name="cc_out")
    nc.gpsimd.collective_compute(
        "AllGather", ALU.bypass,
        replica_groups=[[0, 1, 2, 3], [4, 5, 6, 7]],
        ins=[cc_in[:].opt()], outs=[cc_out[:].opt()])

    # ---- phase D: normalize x1 columns into resident x1n ----
    x1n = [sb.tile([128, L], MMDT, tag=f"bigB{cc}", bufs=1, name=f"x1n{cc}")
           for cc in range(CCH)]
    _norm_full_streamed(nc, sb, ps, x1_e, x1n, ones128, "D")
    x1sh = []
    for cc in range(CCH):
        t = sb.tile([128, SH], MMDT, tag=f"shA{cc}", bufs=1, name=f"x1sh{cc}")
        nc.sync.dma_start(out=t[:], in_=x1s_e[cc*128:(cc+1)*128, :])
        x1sh.append(t)

    # ---- phase F: raw21 columns (fully local; overlaps the all-gather) ----
    for jb in range(8):
        pa = ps.tile([128, L // 2], F32, tag="p1a", name=f"F_pa{jb}")
        pb = ps.tile([128, L // 2], F32, tag="p1b", name=f"F_pb{jb}")
        for n in range(NB):
            dst = pa[:, n*512:(n+1)*512] if n < 4 else pb[:, (n-4)*512:(n-3)*512]
            for cc in range(CCH):
                nc.tensor.matmul(dst,
                                 lhsT=xnn[(cc, jb // 4)][:, (jb % 4)*128:(jb % 4 + 1)*128],
                                 rhs=x1n[cc][:, n*512:(n+1)*512],
                                 start=(cc == 0), stop=(cc == CCH - 1))
        _row_stats(nc, sb, pa, pb, "F", jb, opack_val=opack[3], opack_idx=opack[2])

    # ---- read back gathered x2_new (reuses bigB after x1n's last use) ----
    xg = []
    for cc in range(CCH):
        t = sb.tile([128, L], MMDT, tag=f"bigB{cc}", bufs=1, name=f"xg{cc}")
        for rr in range(4):
            nc.sync.dma_start(out=t[:, rr*SH:(rr+1)*SH],
                              in_=cc_out[rr, cc*128:(cc+1)*128, :])
        xg.append(t)

    # ---- phase E: raw12 rows ----
    for ib in range(8):
        pa = ps.tile([128, L // 2], F32, tag="p1a", name=f"E_pa{ib}")
        pb = ps.tile([128, L // 2], F32, tag="p1b", name=f"E_pb{ib}")
        for n in range(NB):
            dst = pa[:, n*512:(n+1)*512] if n < 4 else pb[:, (n-4)*512:(n-3)*512]
            for cc in range(CCH):
                nc.tensor.matmul(dst,
                                 lhsT=x1sh[cc][:, ib*128:(ib+1)*128],
                                 rhs=xg[cc][:, n*512:(n+1)*512],
                                 start=(cc == 0), stop=(cc == CCH - 1))
        _row_stats(nc, sb, pa, pb, "E", ib, opack_val=opack[1], opack_idx=opack[0])

    # ---- outputs ----
    for k in range(4):
        nc.sync.dma_start(out=out_e[k, :, :], in_=opack[k][:])


_NC_CACHE = None


def _get_nc():
    global _NC_CACHE
    if _NC_CACHE is None:
        _NC_CACHE = build()
    return _NC_CACHE


def _consts():
    c = np.zeros((129, 128), np.float32)
    c[0, :] = 1.0
    c[1:129, :] = np.eye(128, dtype=np.float32)
    return c


def _unpack(v):
    # v: [128, 8] packed [p, b] -> flat vec[b*128 + p]
    return v.T.reshape(-1)


def kernel(x1, x2, gt1, _want_profile=False):
    N = x1.shape[0]
    assert N == 2 and x1.shape == (2, 512, 64, 64)
    x1m = x1.reshape(N, C, L).astype(np.float32)
    x2m = x2.reshape(N, C, L).astype(np.float32)
    gtv = gt1.reshape(N, L)

    in_maps = []
    for core in range(N_CORES):
        g, r = core // 4, core % 4
        sl = slice(r * SH, (r + 1) * SH)
        in_maps.append({
            "x2": np.ascontiguousarray(x2m[g]),
            "x2s": np.ascontiguousarray(x2m[g][:, sl]),
            "x2t": np.ascontiguousarray(x2m[g].T),
            "x1": np.ascontiguousarray(x1m[g]),
            "x1s": np.ascontiguousarray(x1m[g][:, sl]),
            "consts": _consts(),
        })
    nc = _get_nc()
    res = bass_utils.run_bass_kernel_spmd(
        nc, in_maps, core_ids=list(range(N_CORES)),
        trace=_want_profile)

    total = 0.0
    cnt = 0.0
    for g in range(N):
        packs = [res.results[g * 4 + r]["out"] for r in range(4)]
        mid = np.concatenate([_unpack(p[0]) for p in packs]).astype(np.int64)
        assoc = np.concatenate([_unpack(p[1]) for p in packs])
        cidx = np.concatenate([_unpack(p[2]) for p in packs]).astype(np.int64)
        msim = np.concatenate([_unpack(p[3]) for p in packs])
        gtimg = gtv[g]
        indices = cidx[mid]                 # col_argmax[row_argmax]
        reassoc = msim[mid]
        sel = (gtimg == gtimg[indices]) & (gtimg != 255)
        sim = assoc.astype(np.float64) * reassoc.astype(np.float64)
        with np.errstate(divide="ignore"):
            term = np.minimum(-np.log(sim), 100.0)
        total += float((term * sel).sum())
        cnt += float(sel.sum())
    loss = total / max(cnt, 1.0) if cnt > 0 else 0.0
    out = np.float32(loss)
    if _want_profile:
        return out, res
    return out


# revision 28
# speedup vs baseline: 1.5370x; 1.0938x over previous
"""AssociationLoss Trainium2 kernel (8 NeuronCores, SPMD).

Sharding: data-parallel over N=2 images (cores 0-3 -> image 0, cores 4-7 ->
image 1); within an image the [HW, HW] similarity work is sharded row-wise,
1024 rows per core. The spatially-aggregated x2 (spagg) is computed
column-sharded and all-gathered per image group; per-row/column argmax + max
softmax values are produced per shard and the tiny index-chase epilogue
(indices = col_argmax[row_argmax], gt compare, BCE mean over ~16KB/core)
runs on host.

Precision: matmul operands are bf16 (PSUM accumulation fp32). Coherent scale
factors (column norms) are kept fp32 so only incoherent per-element noise
enters the similarity values. Row-standardized softmax scores are invariant to
per-row scaling, so row-side (lhsT) operands are used unnormalized. 1/sqrt is
exp(-0.5*ln(x)) so the whole kernel uses one ACT table set.
"""
import sys, os, json, shutil
sys.path.insert(0, "/opt/trn_rl_repo")
import numpy as np
from concourse import bass, mybir, tile, bacc
from concourse import bass_utils

F32 = mybir.dt.float32
F32R = mybir.dt.float32r
BF16 = mybir.dt.bfloat16
U32 = mybir.dt.uint32
AF = mybir.ActivationFunctionType
ALU = mybir.AluOpType

C = 512
L = 4096
SH = 1024
CCH = 4
NB = L // 512
EPS = 1e-10
N_CORES = 8

MMDT = BF16


_ACT_PATCHED = False


def _ensure_act_json():
    """Steer Bacc's ACT-table-set chooser to natural_log_exp_and_others (it
    contains ln/exp/copy/square — everything this kernel uses) so one
    ACT_TABLE_LOAD suffices instead of per-call set thrashing. Other sets are
    presented as empty (indices preserved for walrus)."""
    # NOTE: steering the chooser by emptying other sets produced wrong ACT
    # results on hardware (runtime table-id mismatch) — disabled.
    return


def build():
    _ensure_act_json()
    nc = bacc.Bacc("TRN2", target_bir_lowering=False, debug=False,
                   num_devices=N_CORES)
    x2_e = nc.declare_dram_parameter("x2", [C, L], MMDT, isOutput=False)
    x2s_e = nc.declare_dram_parameter("x2s", [C, SH], MMDT, isOutput=False)
    x2t_e = nc.declare_dram_parameter("x2t", [L, C], MMDT, isOutput=False)
    x1_e = nc.declare_dram_parameter("x1", [C, L], MMDT, isOutput=False)
    x1s_e = nc.declare_dram_parameter("x1s", [C, SH], MMDT, isOutput=False)
    cst_e = nc.declare_dram_parameter("consts", [129, 128], F32R, isOutput=False)
    out_e = nc.declare_dram_parameter("out", [4, 128, 8], F32, isOutput=True)

    with tile.TileContext(nc) as tc:
        with nc.allow_low_precision(reason="bf16 matmul operands by design"), \
             tc.tile_pool(name="sb", bufs=1) as sb, \
             tc.tile_pool(name="ps", bufs=1, space="PSUM") as ps, \
             tc.tile_pool(name="dr", bufs=1, space="DRAM") as dr:
            _build_body(nc, tc, sb, ps, dr,
                        x2_e, x2s_e, x2t_e, x1_e, x1s_e, cst_e, out_e)
    nc.compile()
    return nc


def _rsqrt_act(nc, sb, dst, src_ap, scale, pref, shape=None):
    """dst = (scale*src)^-0.5 via exp(-0.5*ln(scale*src))."""
    shape = shape or [128, 512]
    lnv = sb.tile(shape, F32, tag="lnscr", bufs=2, name=f"{pref}ln")
    nc.scalar.activation(lnv[:], src_ap, AF.Ln, scale=scale)
    nc.scalar.activation(dst, lnv[:], AF.Exp, scale=-0.5)


def _norm_full_streamed(nc, sb, ps, src_e, out_tiles, ones128b, pref):
    """Column-normalize a [C, L] DRAM tensor into 4x[128, L] bf16 tiles,
    streaming the source twice in [128, 512] chunks. Norm scale stays fp32."""
    npa = ps.tile([128, L // 2], F32, tag="p1a", name=f"{pref}npa")
    npb = ps.tile([128, L // 2], F32, tag="p1b", name=f"{pref}npb")
    nps = [npa[:, n*512:(n+1)*512] for n in range(4)] + \
          [npb[:, n*512:(n+1)*512] for n in range(4)]
    for cc in range(CCH):
        for n in range(NB):
            ld = sb.tile([128, 512], MMDT, tag="bigload", bufs=4,
                         name=f"{pref}ld{cc}_{n}")
            nc.sync.dma_start(out=ld[:], in_=src_e[cc*128:(cc+1)*128,
                                                   n*512:(n+1)*512])
            sq = sb.tile([128, 512], MMDT, tag="sqchunk", bufs=3,
                         name=f"{pref}sq{cc}_{n}")
            nc.scalar.activation(sq[:], ld[:], AF.Square)
            nc.tensor.matmul(nps[n], lhsT=ones128b[:], rhs=sq[:],
                             start=(cc == 0), stop=(cc == CCH - 1))
    for n in range(NB):
        rn = sb.tile([128, 512], F32, tag="rnorm", bufs=2, name=f"{pref}rn{n}")
        _rsqrt_act(nc, sb, rn[:], nps[n], 1.0, f"{pref}rs{n}")
        for cc in range(CCH):
            ld = sb.tile([128, 512], MMDT, tag="bigload", bufs=4,
                         name=f"{pref}ld2{cc}_{n}")
            nc.sync.dma_start(out=ld[:], in_=src_e[cc*128:(cc+1)*128,
                                                   n*512:(n+1)*512])
            nc.vector.tensor_tensor(out_tiles[cc][:, n*512:(n+1)*512],
                                    ld[:], rn[:], ALU.mult)


def _row_stats(nc, sb, p1a, p1b, tag_pref, ib, opack_val, opack_idx):
    """p1a/p1b = [128, 2048] PSUM halves of similarity rows. Standardized
    softmax stats per row; writes argmax and max softmax value to opack."""
    s12 = sb.tile([128, L], F32, tag="srow", bufs=2, name=f"{tag_pref}s{ib}")
    nc.scalar.activation(s12[:, 0:L//2], p1a[:], AF.Copy)
    nc.scalar.activation(s12[:, L//2:L], p1b[:], AF.Copy)
    bno = sb.tile([128, 6 * NB], F32, tag="bno", bufs=2, name=f"{tag_pref}bno{ib}")
    for n in range(4):
        nc.vector.bn_stats(bno[:, n*6:(n+1)*6], p1a[:, n*512:(n+1)*512])
    for n in range(4):
        nc.vector.bn_stats(bno[:, 24+n*6:24+(n+1)*6], p1b[:, n*512:(n+1)*512])
    bna = sb.tile([128, 2], F32, tag="bna", bufs=2, name=f"{tag_pref}bna{ib}")
    nc.vector.bn_aggr(bna[:], bno[:])
    m8 = sb.tile([128, 8], F32, tag="m8", bufs=2, name=f"{tag_pref}m8{ib}")
    nc.vector.max(m8[:], s12[:])
    mi8 = sb.tile([128, 8], U32, tag="mi8", bufs=2, name=f"{tag_pref}mi8{ib}")
    nc.vector.max_index(mi8[:], m8[:], s12[:])
    nc.vector.tensor_copy(opack_idx[:, ib:ib+1], mi8[:, 0:1])

    def tiny(nm):
        return sb.tile([128, 1], F32, tag="tiny", bufs=16, name=f"{tag_pref}{nm}{ib}")
    rinv = tiny("rinv")
    _rsqrt_act(nc, sb, rinv[:], bna[:, 1:2], L / (L - 1.0), f"{tag_pref}ri{ib}",
               shape=[128, 1])
    mr = tiny("mr")
    nc.vector.tensor_tensor(mr[:], bna[:, 0:1], rinv[:], ALU.mult)
    nb_ = tiny("nb")
    nc.vector.tensor_scalar_mul(nb_[:], mr[:], -1.0)
    rse = tiny("rse")
    trash = sb.tile([128, L], BF16, tag="trash", bufs=2, name=f"{tag_pref}t2{ib}")
    nc.scalar.activation(trash[:], s12[:], AF.Exp, scale=rinv[:], bias=nb_[:],
                         accum_out=rse[:])
    t3 = tiny("t3")
    nc.vector.tensor_tensor(t3[:], m8[:, 0:1], rinv[:], ALU.mult)
    t4 = tiny("t4")
    nc.vector.tensor_tensor(t4[:], t3[:], nb_[:], ALU.add)
    em = tiny("em")
    nc.scalar.activation(em[:], t4[:], AF.Exp)
    rr2 = tiny("rr2")
    nc.vector.reciprocal(rr2[:], rse[:])
    nc.vector.tensor_tensor(opack_val[:, ib:ib+1], em[:], rr2[:], ALU.mult)


def _build_body(nc, tc, sb, ps, dr, x2_e, x2s_e, x2t_e, x1_e, x1s_e, cst_e,
                out_e):
    # ---- constants: f32r ones/identity for the stats transpose, bf16 ones
    # for partition-sum matmuls ----
    ones1 = sb.tile([1, 128], F32R, name="ones1")
    nc.sync.dma_start(out=ones1[:], in_=cst_e[0:1, :])
    id128 = sb.tile([128, 128], F32R, name="id128")
    nc.sync.dma_start(out=id128[:], in_=cst_e[1:129, :])
    ones1b = sb.tile([1, 128], BF16, name="ones1b")
    nc.vector.tensor_copy(ones1b[:], ones1[:])
    ones128b = sb.tile([128, 128], BF16, name="ones128b")
    nc.gpsimd.partition_broadcast(ones128b[:], ones1b[:])

    opack = [sb.tile([128, 8], F32, tag=f"op{k}", name=f"opack{k}") for k in range(4)]

    # ---- phase A: normalize x2 columns into resident x2n (bf16) ----
    x2sh = []
    for cc in range(CCH):
        t = sb.tile([128, SH], MMDT, tag=f"shA{cc}", bufs=1, name=f"x2sh{cc}")
        nc.sync.dma_start(out=t[:], in_=x2s_e[cc*128:(cc+1)*128, :])
        x2sh.append(t)
    x2n = [sb.tile([128, L], MMDT, tag=f"bigB{cc}", bufs=1, name=f"x2n{cc}")
           for cc in range(CCH)]
    _norm_full_streamed(nc, sb, ps, x2_e, x2n, ones128b, "A")

    # ---- phase B: sim_self row stats (raw lhsT; row-scale invariant) ----
    spack = sb.tile([128, 16], F32R, name="spack")
    for ib in range(8):
        pa = ps.tile([128, L // 2], F32, tag="p1a", name=f"B_pa{ib}")
        pb = ps.tile([128, L // 2], F32, tag="p1b", name=f"B_pb{ib}")
        for n in range(NB):
            dst = pa[:, n*512:(n+1)*512] if n < 4 else pb[:, (n-4)*512:(n-3)*512]
            for cc in range(CCH):
                nc.tensor.matmul(dst,
                                 lhsT=x2sh[cc][:, ib*128:(ib+1)*128],
                                 rhs=x2n[cc][:, n*512:(n+1)*512],
                                 start=(cc == 0), stop=(cc == CCH - 1))
        bno = sb.tile([128, 6 * NB], F32, tag="bno", bufs=2, name=f"Bbno{ib}")
        for n in range(4):
            nc.vector.bn_stats(bno[:, n*6:(n+1)*6], pa[:, n*512:(n+1)*512])
        for n in range(4):
            nc.vector.bn_stats(bno[:, 24+n*6:24+(n+1)*6], pb[:, n*512:(n+1)*512])
        bna = sb.tile([128, 2], F32, tag="bna", bufs=2, name=f"Bbna{ib}")
        nc.vector.bn_aggr(bna[:], bno[:])
        rinv = sb.tile([128, 1], F32, tag="tiny", bufs=16, name=f"Brinv{ib}")
        _rsqrt_act(nc, sb, rinv[:], bna[:, 1:2], L / (L - 1.0), f"Bri{ib}",
                   shape=[128, 1])
        nc.vector.tensor_copy(spack[:, ib:ib+1], rinv[:])
        mr = sb.tile([128, 1], F32, tag="tiny", bufs=16, name=f"Bmr{ib}")
        nc.vector.tensor_tensor(mr[:], bna[:, 0:1], rinv[:], ALU.mult)
        nc.vector.tensor_scalar_mul(spack[:, 8+ib:9+ib], mr[:], -1.0)
    stT = ps.tile([16, 128], F32R, tag="p1a", name="stT")
    nc.tensor.transpose(stT[:], spack[:], id128[:])
    st = sb.tile([16, 128], F32R, name="st")
    nc.vector.tensor_copy(st[:], stT[:])
    # move rinv/bias rows to partition 0 as [1, 1024] rows (DRAM bounce)
    b_dram = dr.tile([1, SH], F32R, name="b_dram")
    nc.sync.dma_start(out=b_dram[:, :].rearrange("o (a b) -> (o a) b", a=8),
                      in_=st[8:16, :])
    b_rowf = sb.tile([1, SH], F32R, name="b_rowf")
    nc.sync.dma_start(out=b_rowf[:], in_=b_dram[:, :])
    b_row = sb.tile([1, SH], BF16, name="b_row")
    nc.vector.tensor_copy(b_row[:], b_rowf[:])
    r_dram = dr.tile([1, SH], F32R, name="r_dram")
    nc.sync.dma_start(out=r_dram[:, :].rearrange("o (a b) -> (o a) b", a=8),
                      in_=st[0:8, :])
    r_rowf = sb.tile([1, SH], F32R, name="r_rowf")
    nc.sync.dma_start(out=r_rowf[:], in_=r_dram[:, :])

    # ---- phase C: sim_self exp + agg + spagg combine + renormalize ----
    cc_in = dr.tile([C, SH], MMDT, name="cc_in")
    xnn = {}
    for ic in range(2):
        rbc = sb.tile([128, 512], F32R, tag="rbc", bufs=1, name=f"rbc{ic}")
        nc.gpsimd.partition_broadcast(rbc[:], r_rowf[0:1, ic*512:(ic+1)*512])
        ys = []
        for cc in range(CCH):
            y = sb.tile([128, 512], MMDT, tag=f"y{cc}", bufs=1, name=f"y{cc}_{ic}")
            nc.vector.tensor_tensor(y[:], x2sh[cc][:, ic*512:(ic+1)*512], rbc[:],
                                    ALU.mult)
            ys.append(y)
        pa = ps.tile([128, L // 2], F32, tag="p1a", name=f"C_pa{ic}")
        pb = ps.tile([128, L // 2], F32, tag="p1b", name=f"C_pb{ic}")
        # bank map: pa = [zt0, zt1, agg0, agg1]; pb = [agg2, agg3, rse, np2]
        agg_ps = [pa[:, 1024:1536], pa[:, 1536:2048], pb[:, 0:512], pb[:, 512:1024]]
        rse_ps = pb[0:1, 1024:1536]
        for jb in range(32):
            zt = pa[:, (jb % 2)*512:(jb % 2 + 1)*512]
            for cc in range(CCH):
                nc.tensor.matmul(zt, lhsT=x2n[cc][:, jb*128:(jb+1)*128],
                                 rhs=ys[cc][:], start=(cc == 0), stop=False)
            nc.tensor.matmul(zt, lhsT=ones1b[:],
                             rhs=b_row[0:1, ic*512:(ic+1)*512],
                             start=False, stop=True)
            ez = sb.tile([128, 512], MMDT, tag="ez", bufs=3, name=f"ez{ic}_{jb}")
            nc.scalar.activation(ez[:], zt, AF.Exp)
            xt = sb.tile([128, C], MMDT, tag="xt", bufs=3, name=f"xt{ic}_{jb}")
            nc.sync.dma_start(out=xt[:], in_=x2t_e[jb*128:(jb+1)*128, :])
            for cc in range(CCH):
                nc.tensor.matmul(agg_ps[cc], lhsT=xt[:, cc*128:(cc+1)*128],
                                 rhs=ez[:], start=(jb == 0), stop=(jb == 31))
            nc.tensor.matmul(rse_ps, lhsT=ones128b[:, 0:1], rhs=ez[:],
                             start=(jb == 0), stop=(jb == 31))
        rrse = sb.tile([1, 512], F32, tag="rrse", bufs=1, name=f"rrse{ic}")
        nc.vector.reciprocal(rrse[:], rse_ps)
        rrb = sb.tile([128, 512], F32, tag="rrb", bufs=1, name=f"rrb{ic}")
        nc.gpsimd.partition_broadcast(rrb[:], rrse[:])
        nps2 = pb[:, 1536:2048]
        xss_l = []
        for cc in range(CCH):
            tmp = sb.tile([128, 512], F32, tag="tmpc", bufs=1, name=f"tmp{cc}_{ic}")
            nc.vector.tensor_tensor(tmp[:], agg_ps[cc], rrb[:], ALU.mult)
            xss = sb.tile([128, 512], F32, tag="xss", bufs=4, name=f"xss{cc}_{ic}")
            nc.vector.tensor_tensor(xss[:], tmp[:], x2sh[cc][:, ic*512:(ic+1)*512],
                                    ALU.add)
            xss_l.append(xss)
            sq2 = sb.tile([128, 512], MMDT, tag="sqchunk", bufs=3,
                          name=f"sq2{cc}_{ic}")
            nc.scalar.activation(sq2[:], xss[:], AF.Square)
            nc.tensor.matmul(nps2, lhsT=ones128b[:], rhs=sq2[:],
                             start=(cc == 0), stop=(cc == CCH - 1))
        rnb2 = sb.tile([128, 512], F32, tag="rnb2", bufs=1, name=f"rnb2{ic}")
        _rsqrt_act(nc, sb, rnb2[:], nps2, 1.0, f"rnb2s{ic}")
        for cc in range(CCH):
            xo = sb.tile([128, 512], MMDT, tag="xnn", bufs=8, name=f"xnn{cc}_{ic}")
            nc.vector.tensor_tensor(xo[:], xss_l[cc][:], rnb2[:], ALU.mult)
            xnn[(cc, ic)] = xo
            nc.sync.dma_start(out=cc_in[cc*128:(cc+1)*128, ic*512:(ic+1)*512],
                              in_=xo[:])

    # ---- all-gather normalized x2_new within each image group ----
    cc_out = dr.tile([4, C, SH], MMDT, name="cc_out")
    nc.gpsimd.collective_compute(
        "AllGather", ALU.bypass,
        replica_groups=[[0, 1, 2, 3], [4, 5, 6, 7]],
        ins=[cc_in[:].opt()], outs=[cc_out[:].opt()])

    # ---- phase D: normalize x1 columns into resident x1n ----
    x1n = [sb.tile([128, L], MMDT, tag=f"bigB{cc}", bufs=1, name=f"x1n{cc}")
           for cc in range(CCH)]
    _norm_full_streamed(nc, sb, ps, x1_e, x1n, ones128b, "D")
    x1sh = []
    for cc in range(CCH):
        t = sb.tile([128, SH], MMDT, tag=f"shA{cc}", bufs=1, name=f"x1sh{cc}")
        nc.sync.dma_start(out=t[:], in_=x1s_e[cc*128:(cc+1)*128, :])
        x1sh.append(t)

    # ---- phase F: raw21 columns (fully local; overlaps the all-gather) ----
    for jb in range(8):
        pa = ps.tile([128, L // 2], F32, tag="p1a", name=f"F_pa{jb}")
        pb = ps.tile([128, L // 2], F32, tag="p1b", name=f"F_pb{jb}")
        for n in range(NB):
            dst = pa[:, n*512:(n+1)*512] if n < 4 else pb[:, (n-4)*512:(n-3)*512]
            for cc in range(CCH):
                nc.tensor.matmul(dst,
                                 lhsT=xnn[(cc, jb // 4)][:, (jb % 4)*128:(jb % 4 + 1)*128],
                                 rhs=x1n[cc][:, n*512:(n+1)*512],
                                 start=(cc == 0), stop=(cc == CCH - 1))
        _row_stats(nc, sb, pa, pb, "F", jb, opack_val=opack[3], opack_idx=opack[2])

    # ---- read back gathered x2_new (reuses bigB after x1n's last use) ----
    xg = []
    for cc in range(CCH):
        t = sb.tile([128, L], MMDT, tag=f"bigB{cc}", bufs=1, name=f"xg{cc}")
        for rr in range(4):
            nc.sync.dma_start(out=t[:, rr*SH:(rr+1)*SH],
                              in_=cc_out[rr, cc*128:(cc+1)*128, :])
        xg.append(t)

    # ---- phase E: raw12 rows ----
    for ib in range(8):
        pa = ps.tile([128, L // 2], F32, tag="p1a", name=f"E_pa{ib}")
        pb = ps.tile([128, L // 2], F32, tag="p1b", name=f"E_pb{ib}")
        for n in range(NB):
            dst = pa[:, n*512:(n+1)*512] if n < 4 else pb[:, (n-4)*512:(n-3)*512]
            for cc in range(CCH):
                nc.tensor.matmul(dst,
                                 lhsT=x1sh[cc][:, ib*128:(ib+1)*128],
                                 rhs=xg[cc][:, n*512:(n+1)*512],
                                 start=(cc == 0), stop=(cc == CCH - 1))
        _row_stats(nc, sb, pa, pb, "E", ib, opack_val=opack[1], opack_idx=opack[0])

    # ---- outputs ----
    for k in range(4):
        nc.sync.dma_start(out=out_e[k, :, :], in_=opack[k][:])


_NC_CACHE = None


def _get_nc():
    global _NC_CACHE
    if _NC_CACHE is None:
        _NC_CACHE = build()
    return _NC_CACHE


def _consts():
    c = np.zeros((129, 128), np.float32)
    c[0, :] = 1.0
    c[1:129, :] = np.eye(128, dtype=np.float32)
    return c


def _unpack(v):
    # v: [128, 8] packed [p, b] -> flat vec[b*128 + p]
    return v.T.reshape(-1)


def kernel(x1, x2, gt1, _want_profile=False):
    N = x1.shape[0]
    assert N == 2 and x1.shape == (2, 512, 64, 64)
    bf = mybir.dt.np(BF16)
    x1m = x1.reshape(N, C, L).astype(np.float32)
    x2m = x2.reshape(N, C, L).astype(np.float32)
    gtv = gt1.reshape(N, L)

    in_maps = []
    for core in range(N_CORES):
        g, r = core // 4, core % 4
        sl = slice(r * SH, (r + 1) * SH)
        in_maps.append({
            "x2": np.ascontiguousarray(x2m[g]).astype(bf),
            "x2s": np.ascontiguousarray(x2m[g][:, sl]).astype(bf),
            "x2t": np.ascontiguousarray(x2m[g].T).astype(bf),
            "x1": np.ascontiguousarray(x1m[g]).astype(bf),
            "x1s": np.ascontiguousarray(x1m[g][:, sl]).astype(bf),
            "consts": _consts(),
        })
    nc = _get_nc()
    res = bass_utils.run_bass_kernel_spmd(
        nc, in_maps, core_ids=list(range(N_CORES)),
        trace=_want_profile)

    total = 0.0
    cnt = 0.0
    for g in range(N):
        packs = [res.results[g * 4 + r]["out"] for r in range(4)]
        mid = np.concatenate([_unpack(p[0]) for p in packs]).astype(np.int64)
        assoc = np.concatenate([_unpack(p[1]) for p in packs])
        cidx = np.concatenate([_unpack(p[2]) for p in packs]).astype(np.int64)
        msim = np.concatenate([_unpack(p[3]) for p in packs])
        gtimg = gtv[g]
        indices = cidx[mid]
        reassoc = msim[mid]
        sel = (gtimg == gtimg[indices]) & (gtimg != 255)
        sim = assoc.astype(np.float64) * reassoc.astype(np.float64)
        with np.errstate(divide="ignore"):
            term = np.minimum(-np.log(sim), 100.0)
        total += float((term * sel).sum())
        cnt += float(sel.sum())
    loss = total / max(cnt, 1.0) if cnt > 0 else 0.0
    out = np.float32(loss)
    if _want_profile:
        return out, res
    return out


# revision 29
# speedup vs baseline: 1.7681x; 1.1504x over previous
"""AssociationLoss Trainium2 kernel (8 NeuronCores, SPMD).

Sharding: data-parallel over N=2 images (cores 0-3 -> image 0, cores 4-7 ->
image 1); within an image the [HW, HW] similarity work is sharded row-wise,
1024 rows per core. The spatially-aggregated x2 (spagg) is computed
column-sharded and all-gathered per image group; per-row/column argmax + max
softmax values are produced per shard and the tiny index-chase epilogue
(indices = col_argmax[row_argmax], gt compare, BCE mean over ~16KB/core)
runs on host.

Precision: matmul operands are bf16 (PSUM accumulation fp32). Coherent scale
factors (column norms) are kept fp32 so only incoherent per-element noise
enters the similarity values. Row-standardized softmax scores are invariant to
per-row scaling, so row-side (lhsT) operands are used unnormalized. 1/sqrt is
exp(-0.5*ln(x)) so the whole kernel uses one ACT table set.
"""
import sys, os, json, shutil
sys.path.insert(0, "/opt/trn_rl_repo")
import numpy as np
from concourse import bass, mybir, tile, bacc
from concourse import bass_utils

F32 = mybir.dt.float32
F32R = mybir.dt.float32r
BF16 = mybir.dt.bfloat16
U32 = mybir.dt.uint32
AF = mybir.ActivationFunctionType
ALU = mybir.AluOpType

C = 512
L = 4096
SH = 1024
CCH = 4
NB = L // 512
EPS = 1e-10
N_CORES = 8

MMDT = BF16


_ACT_PATCHED = False


def _ensure_act_json():
    """Steer Bacc's ACT-table-set chooser to natural_log_exp_and_others (it
    contains ln/exp/copy/square — everything this kernel uses) so one
    ACT_TABLE_LOAD suffices instead of per-call set thrashing. Other sets are
    presented as empty (indices preserved for walrus)."""
    # NOTE: steering the chooser by emptying other sets produced wrong ACT
    # results on hardware (runtime table-id mismatch) — disabled.
    return


def build():
    _ensure_act_json()
    nc = bacc.Bacc("TRN2", target_bir_lowering=False, debug=False,
                   num_devices=N_CORES)
    x2_e = nc.declare_dram_parameter("x2", [C, L], MMDT, isOutput=False)
    x2s_e = nc.declare_dram_parameter("x2s", [C, SH], MMDT, isOutput=False)
    x2t_e = nc.declare_dram_parameter("x2t", [L, C], MMDT, isOutput=False)
    x1_e = nc.declare_dram_parameter("x1", [C, L], MMDT, isOutput=False)
    x1s_e = nc.declare_dram_parameter("x1s", [C, SH], MMDT, isOutput=False)
    cst_e = nc.declare_dram_parameter("consts", [129, 128], F32R, isOutput=False)
    out_e = nc.declare_dram_parameter("out", [4, 128, 8], F32, isOutput=True)

    with tile.TileContext(nc) as tc:
        with nc.allow_low_precision(reason="bf16 matmul operands by design"), \
             tc.tile_pool(name="sb", bufs=1) as sb, \
             tc.tile_pool(name="ps", bufs=1, space="PSUM") as ps, \
             tc.tile_pool(name="dr", bufs=1, space="DRAM") as dr:
            _build_body(nc, tc, sb, ps, dr,
                        x2_e, x2s_e, x2t_e, x1_e, x1s_e, cst_e, out_e)
    nc.compile()
    return nc


def _rsqrt_act(nc, sb, dst, src_ap, scale, pref, shape=None):
    """dst = (scale*src)^-0.5 via exp(-0.5*ln(scale*src))."""
    shape = shape or [128, 512]
    lnv = sb.tile(shape, F32, tag="lnscr", bufs=2, name=f"{pref}ln")
    nc.scalar.activation(lnv[:], src_ap, AF.Ln, scale=scale)
    nc.scalar.activation(dst, lnv[:], AF.Exp, scale=-0.5)


def _norm_full_streamed(nc, sb, ps, src_e, out_tiles, ones128b, pref):
    """Column-normalize a [C, L] DRAM tensor into 4x[128, L] bf16 tiles,
    streaming the source twice in [128, 512] chunks. Norm scale stays fp32."""
    npq = [ps.tile([128, SH], F32, tag=f"q{q}", name=f"{pref}npq{q}")
           for q in range(4)]
    nps = [npq[n // 2][:, (n % 2)*512:(n % 2 + 1)*512] for n in range(NB)]
    for cc in range(CCH):
        for n in range(NB):
            ld = sb.tile([128, 512], MMDT, tag="bigload", bufs=4,
                         name=f"{pref}ld{cc}_{n}")
            nc.sync.dma_start(out=ld[:], in_=src_e[cc*128:(cc+1)*128,
                                                   n*512:(n+1)*512])
            sq = sb.tile([128, 512], MMDT, tag="sqchunk", bufs=3,
                         name=f"{pref}sq{cc}_{n}")
            nc.scalar.activation(sq[:], ld[:], AF.Square)
            nc.tensor.matmul(nps[n], lhsT=ones128b[:], rhs=sq[:],
                             start=(cc == 0), stop=(cc == CCH - 1))
    for n in range(NB):
        rn = sb.tile([128, 512], F32, tag="rnorm", bufs=2, name=f"{pref}rn{n}")
        _rsqrt_act(nc, sb, rn[:], nps[n], 1.0, f"{pref}rs{n}")
        for cc in range(CCH):
            ld = sb.tile([128, 512], MMDT, tag="bigload", bufs=4,
                         name=f"{pref}ld2{cc}_{n}")
            nc.sync.dma_start(out=ld[:], in_=src_e[cc*128:(cc+1)*128,
                                                   n*512:(n+1)*512])
            nc.vector.tensor_tensor(out_tiles[cc][:, n*512:(n+1)*512],
                                    ld[:], rn[:], ALU.mult)


def _row_stats(nc, sb, quarters, tag_pref, ib, opack_val, opack_idx):
    """quarters = 4x [128, 1024] PSUM quarters of similarity rows.
    Standardized softmax stats per row; argmax + max softmax value to opack."""
    s12 = sb.tile([128, L], F32, tag="srow", bufs=2, name=f"{tag_pref}s{ib}")
    bno = sb.tile([128, 6 * NB], F32, tag="bno", bufs=2, name=f"{tag_pref}bno{ib}")
    for q in range(4):
        nc.scalar.activation(s12[:, q*SH:(q+1)*SH], quarters[q][:], AF.Copy)
        for h in range(2):
            n = q*2 + h
            nc.vector.bn_stats(bno[:, n*6:(n+1)*6],
                               quarters[q][:, h*512:(h+1)*512])
    bna = sb.tile([128, 2], F32, tag="bna", bufs=2, name=f"{tag_pref}bna{ib}")
    nc.vector.bn_aggr(bna[:], bno[:])
    m8 = sb.tile([128, 8], F32, tag="m8", bufs=2, name=f"{tag_pref}m8{ib}")
    nc.vector.max(m8[:], s12[:])
    mi8 = sb.tile([128, 8], U32, tag="mi8", bufs=2, name=f"{tag_pref}mi8{ib}")
    nc.vector.max_index(mi8[:], m8[:], s12[:])
    nc.vector.tensor_copy(opack_idx[:, ib:ib+1], mi8[:, 0:1])

    def tiny(nm):
        return sb.tile([128, 1], F32, tag="tiny", bufs=16, name=f"{tag_pref}{nm}{ib}")
    rinv = tiny("rinv")
    _rsqrt_act(nc, sb, rinv[:], bna[:, 1:2], L / (L - 1.0), f"{tag_pref}ri{ib}",
               shape=[128, 1])
    mr = tiny("mr")
    nc.vector.tensor_tensor(mr[:], bna[:, 0:1], rinv[:], ALU.mult)
    nb_ = tiny("nb")
    nc.vector.tensor_scalar_mul(nb_[:], mr[:], -1.0)
    rse = tiny("rse")
    trash = sb.tile([128, L], BF16, tag="trash", bufs=2, name=f"{tag_pref}t2{ib}")
    nc.scalar.activation(trash[:], s12[:], AF.Exp, scale=rinv[:], bias=nb_[:],
                         accum_out=rse[:])
    t3 = tiny("t3")
    nc.vector.tensor_tensor(t3[:], m8[:, 0:1], rinv[:], ALU.mult)
    t4 = tiny("t4")
    nc.vector.tensor_tensor(t4[:], t3[:], nb_[:], ALU.add)
    em = tiny("em")
    nc.scalar.activation(em[:], t4[:], AF.Exp)
    rr2 = tiny("rr2")
    nc.vector.reciprocal(rr2[:], rse[:])
    nc.vector.tensor_tensor(opack_val[:, ib:ib+1], em[:], rr2[:], ALU.mult)


def _build_body(nc, tc, sb, ps, dr, x2_e, x2s_e, x2t_e, x1_e, x1s_e, cst_e,
                out_e):
    # ---- constants: f32r ones/identity for the stats transpose, bf16 ones
    # for partition-sum matmuls ----
    ones1 = sb.tile([1, 128], F32R, name="ones1")
    nc.sync.dma_start(out=ones1[:], in_=cst_e[0:1, :])
    id128 = sb.tile([128, 128], F32R, name="id128")
    nc.sync.dma_start(out=id128[:], in_=cst_e[1:129, :])
    ones1b = sb.tile([1, 128], BF16, name="ones1b")
    nc.vector.tensor_copy(ones1b[:], ones1[:])
    ones128b = sb.tile([128, 128], BF16, name="ones128b")
    nc.gpsimd.partition_broadcast(ones128b[:], ones1b[:])

    opack = [sb.tile([128, 8], F32, tag=f"op{k}", name=f"opack{k}") for k in range(4)]

    # ---- phase A: normalize x2 columns into resident x2n (bf16) ----
    x2sh = []
    for cc in range(CCH):
        t = sb.tile([128, SH], MMDT, tag=f"shA{cc}", bufs=1, name=f"x2sh{cc}")
        nc.sync.dma_start(out=t[:], in_=x2s_e[cc*128:(cc+1)*128, :])
        x2sh.append(t)
    x2n = [sb.tile([128, L], MMDT, tag=f"bigB{cc}", bufs=1, name=f"x2n{cc}")
           for cc in range(CCH)]
    _norm_full_streamed(nc, sb, ps, x2_e, x2n, ones128b, "A")

    # ---- phase B: sim_self row stats (raw lhsT; row-scale invariant) ----
    spack = sb.tile([128, 16], F32R, name="spack")
    for ib in range(8):
        qs = [ps.tile([128, SH], F32, tag=f"q{q}", name=f"B_q{q}_{ib}")
              for q in range(4)]
        for q in range(4):
            for cc in range(CCH):
                for h in range(2):
                    n = q*2 + h
                    nc.tensor.matmul(qs[q][:, h*512:(h+1)*512],
                                     lhsT=x2sh[cc][:, ib*128:(ib+1)*128],
                                     rhs=x2n[cc][:, n*512:(n+1)*512],
                                     start=(cc == 0), stop=(cc == CCH - 1))
        bno = sb.tile([128, 6 * NB], F32, tag="bno", bufs=2, name=f"Bbno{ib}")
        for q in range(4):
            for h in range(2):
                n = q*2 + h
                nc.vector.bn_stats(bno[:, n*6:(n+1)*6],
                                   qs[q][:, h*512:(h+1)*512])
        bna = sb.tile([128, 2], F32, tag="bna", bufs=2, name=f"Bbna{ib}")
        nc.vector.bn_aggr(bna[:], bno[:])
        rinv = sb.tile([128, 1], F32, tag="tiny", bufs=16, name=f"Brinv{ib}")
        _rsqrt_act(nc, sb, rinv[:], bna[:, 1:2], L / (L - 1.0), f"Bri{ib}",
                   shape=[128, 1])
        nc.vector.tensor_copy(spack[:, ib:ib+1], rinv[:])
        mr = sb.tile([128, 1], F32, tag="tiny", bufs=16, name=f"Bmr{ib}")
        nc.vector.tensor_tensor(mr[:], bna[:, 0:1], rinv[:], ALU.mult)
        nc.vector.tensor_scalar_mul(spack[:, 8+ib:9+ib], mr[:], -1.0)
    stT = ps.tile([16, 128], F32R, tag="q0", name="stT")
    nc.tensor.transpose(stT[:], spack[:], id128[:])
    st = sb.tile([16, 128], F32R, name="st")
    nc.vector.tensor_copy(st[:], stT[:])
    # move rinv/bias rows to partition 0 as [1, 1024] rows (DRAM bounce)
    b_dram = dr.tile([1, SH], F32R, name="b_dram")
    nc.sync.dma_start(out=b_dram[:, :].rearrange("o (a b) -> (o a) b", a=8),
                      in_=st[8:16, :])
    b_rowf = sb.tile([1, SH], F32R, name="b_rowf")
    nc.sync.dma_start(out=b_rowf[:], in_=b_dram[:, :])
    b_row = sb.tile([1, SH], BF16, name="b_row")
    nc.vector.tensor_copy(b_row[:], b_rowf[:])
    r_dram = dr.tile([1, SH], F32R, name="r_dram")
    nc.sync.dma_start(out=r_dram[:, :].rearrange("o (a b) -> (o a) b", a=8),
                      in_=st[0:8, :])
    r_rowf = sb.tile([1, SH], F32R, name="r_rowf")
    nc.sync.dma_start(out=r_rowf[:], in_=r_dram[:, :])

    # ---- phase C: sim_self exp + agg + spagg combine + renormalize ----
    cc_in = dr.tile([C, SH], MMDT, name="cc_in")
    xnn = {}
    for ic in range(2):
        rbc = sb.tile([128, 512], F32R, tag="rbc", bufs=1, name=f"rbc{ic}")
        nc.gpsimd.partition_broadcast(rbc[:], r_rowf[0:1, ic*512:(ic+1)*512])
        ys = []
        for cc in range(CCH):
            y = sb.tile([128, 512], MMDT, tag=f"y{cc}", bufs=1, name=f"y{cc}_{ic}")
            nc.vector.tensor_tensor(y[:], x2sh[cc][:, ic*512:(ic+1)*512], rbc[:],
                                    ALU.mult)
            ys.append(y)
        q0 = ps.tile([128, SH], F32, tag="q0", name=f"C_q0_{ic}")
        q1 = ps.tile([128, SH], F32, tag="q1", name=f"C_q1_{ic}")
        q2 = ps.tile([128, SH], F32, tag="q2", name=f"C_q2_{ic}")
        q3 = ps.tile([128, SH], F32, tag="q3", name=f"C_q3_{ic}")
        # bank map: q0 = [zt0, zt1]; q1/q2 = agg; q3 = [rse, np2]
        pa = q0
        agg_ps = [q1[:, 0:512], q1[:, 512:1024], q2[:, 0:512], q2[:, 512:1024]]
        rse_ps = q3[0:1, 0:512]
        for jb in range(32):
            zt = pa[:, (jb % 2)*512:(jb % 2 + 1)*512]
            for cc in range(CCH):
                nc.tensor.matmul(zt, lhsT=x2n[cc][:, jb*128:(jb+1)*128],
                                 rhs=ys[cc][:], start=(cc == 0), stop=False)
            nc.tensor.matmul(zt, lhsT=ones1b[:],
                             rhs=b_row[0:1, ic*512:(ic+1)*512],
                             start=False, stop=True)
            ez = sb.tile([128, 512], MMDT, tag="ez", bufs=3, name=f"ez{ic}_{jb}")
            nc.scalar.activation(ez[:], zt, AF.Exp)
            xt = sb.tile([128, C], MMDT, tag="xt", bufs=3, name=f"xt{ic}_{jb}")
            nc.sync.dma_start(out=xt[:], in_=x2t_e[jb*128:(jb+1)*128, :])
            for cc in range(CCH):
                nc.tensor.matmul(agg_ps[cc], lhsT=xt[:, cc*128:(cc+1)*128],
                                 rhs=ez[:], start=(jb == 0), stop=(jb == 31))
            nc.tensor.matmul(rse_ps, lhsT=ones128b[:, 0:1], rhs=ez[:],
                             start=(jb == 0), stop=(jb == 31))
        rrse = sb.tile([1, 512], F32, tag="rrse", bufs=1, name=f"rrse{ic}")
        nc.vector.reciprocal(rrse[:], rse_ps)
        rrb = sb.tile([128, 512], F32, tag="rrb", bufs=1, name=f"rrb{ic}")
        nc.gpsimd.partition_broadcast(rrb[:], rrse[:])
        nps2 = q3[:, 512:1024]
        xss_l = []
        for cc in range(CCH):
            tmp = sb.tile([128, 512], F32, tag="tmpc", bufs=1, name=f"tmp{cc}_{ic}")
            nc.vector.tensor_tensor(tmp[:], agg_ps[cc], rrb[:], ALU.mult)
            xss = sb.tile([128, 512], F32, tag="xss", bufs=4, name=f"xss{cc}_{ic}")
            nc.vector.tensor_tensor(xss[:], tmp[:], x2sh[cc][:, ic*512:(ic+1)*512],
                                    ALU.add)
            xss_l.append(xss)
            sq2 = sb.tile([128, 512], MMDT, tag="sqchunk", bufs=3,
                          name=f"sq2{cc}_{ic}")
            nc.scalar.activation(sq2[:], xss[:], AF.Square)
            nc.tensor.matmul(nps2, lhsT=ones128b[:], rhs=sq2[:],
                             start=(cc == 0), stop=(cc == CCH - 1))
        rnb2 = sb.tile([128, 512], F32, tag="rnb2", bufs=1, name=f"rnb2{ic}")
        _rsqrt_act(nc, sb, rnb2[:], nps2, 1.0, f"rnb2s{ic}")
        for cc in range(CCH):
            xo = sb.tile([128, 512], MMDT, tag="xnn", bufs=8, name=f"xnn{cc}_{ic}")
            nc.vector.tensor_tensor(xo[:], xss_l[cc][:], rnb2[:], ALU.mult)
            xnn[(cc, ic)] = xo
            nc.sync.dma_start(out=cc_in[cc*128:(cc+1)*128, ic*512:(ic+1)*512],
                              in_=xo[:])

    # ---- all-gather normalized x2_new within each image group ----
    cc_out = dr.tile([4, C, SH], MMDT, name="cc_out")
    nc.gpsimd.collective_compute(
        "AllGather", ALU.bypass,
        replica_groups=[[0, 1, 2, 3], [4, 5, 6, 7]],
        ins=[cc_in[:].opt()], outs=[cc_out[:].opt()])

    # ---- phase D: normalize x1 columns into resident x1n ----
    x1n = [sb.tile([128, L], MMDT, tag=f"bigB{cc}", bufs=1, name=f"x1n{cc}")
           for cc in range(CCH)]
    _norm_full_streamed(nc, sb, ps, x1_e, x1n, ones128b, "D")
    x1sh = []
    for cc in range(CCH):
        t = sb.tile([128, SH], MMDT, tag=f"shA{cc}", bufs=1, name=f"x1sh{cc}")
        nc.sync.dma_start(out=t[:], in_=x1s_e[cc*128:(cc+1)*128, :])
        x1sh.append(t)

    # ---- phase F: raw21 columns (fully local; overlaps the all-gather) ----
    for jb in range(8):
        qs = [ps.tile([128, SH], F32, tag=f"q{q}", name=f"F_q{q}_{jb}")
              for q in range(4)]
        for q in range(4):
            for cc in range(CCH):
                for h in range(2):
                    n = q*2 + h
                    nc.tensor.matmul(qs[q][:, h*512:(h+1)*512],
                                     lhsT=xnn[(cc, jb // 4)][:, (jb % 4)*128:(jb % 4 + 1)*128],
                                     rhs=x1n[cc][:, n*512:(n+1)*512],
                                     start=(cc == 0), stop=(cc == CCH - 1))
        _row_stats(nc, sb, qs, "F", jb, opack_val=opack[3], opack_idx=opack[2])

    # ---- read back gathered x2_new (reuses bigB after x1n's last use) ----
    xg = []
    for cc in range(CCH):
        t = sb.tile([128, L], MMDT, tag=f"bigB{cc}", bufs=1, name=f"xg{cc}")
        for rr in range(4):
            nc.sync.dma_start(out=t[:, rr*SH:(rr+1)*SH],
                              in_=cc_out[rr, cc*128:(cc+1)*128, :])
        xg.append(t)

    # ---- phase E: raw12 rows ----
    for ib in range(8):
        qs = [ps.tile([128, SH], F32, tag=f"q{q}", name=f"E_q{q}_{ib}")
              for q in range(4)]
        for q in range(4):
            for cc in range(CCH):
                for h in range(2):
                    n = q*2 + h
                    nc.tensor.matmul(qs[q][:, h*512:(h+1)*512],
                                     lhsT=x1sh[cc][:, ib*128:(ib+1)*128],
                                     rhs=xg[cc][:, n*512:(n+1)*512],
                                     start=(cc == 0), stop=(cc == CCH - 1))
        _row_stats(nc, sb, qs, "E", ib, opack_val=opack[1], opack_idx=opack[0])

    # ---- outputs ----
    for k in range(4):
        nc.sync.dma_start(out=out_e[k, :, :], in_=opack[k][:])


_NC_CACHE = None


def _get_nc():
    global _NC_CACHE
    if _NC_CACHE is None:
        _NC_CACHE = build()
    return _NC_CACHE


def _consts():
    c = np.zeros((129, 128), np.float32)
    c[0, :] = 1.0
    c[1:129, :] = np.eye(128, dtype=np.float32)
    return c


def _unpack(v):
    # v: [128, 8] packed [p, b] -> flat vec[b*128 + p]
    return v.T.reshape(-1)


def kernel(x1, x2, gt1, _want_profile=False):
    N = x1.shape[0]
    assert N == 2 and x1.shape == (2, 512, 64, 64)
    bf = mybir.dt.np(BF16)
    x1m = x1.reshape(N, C, L).astype(np.float32)
    x2m = x2.reshape(N, C, L).astype(np.float32)
    gtv = gt1.reshape(N, L)

    in_maps = []
    for core in range(N_CORES):
        g, r = core // 4, core % 4
        sl = slice(r * SH, (r + 1) * SH)
        in_maps.append({
            "x2": np.ascontiguousarray(x2m[g]).astype(bf),
            "x2s": np.ascontiguousarray(x2m[g][:, sl]).astype(bf),
            "x2t": np.ascontiguousarray(x2m[g].T).astype(bf),
            "x1": np.ascontiguousarray(x1m[g]).astype(bf),
            "x1s": np.ascontiguousarray(x1m[g][:, sl]).astype(bf),
            "consts": _consts(),
        })
    nc = _get_nc()
    res = bass_utils.run_bass_kernel_spmd(
        nc, in_maps, core_ids=list(range(N_CORES)),
        trace=_want_profile)

    total = 0.0
    cnt = 0.0
    for g in range(N):
        packs = [res.results[g * 4 + r]["out"] for r in range(4)]
        mid = np.concatenate([_unpack(p[0]) for p in packs]).astype(np.int64)
        assoc = np.concatenate([_unpack(p[1]) for p in packs])
        cidx = np.concatenate([_unpack(p[2]) for p in packs]).astype(np.int64)
        msim = np.concatenate([_unpack(p[3]) for p in packs])
        gtimg = gtv[g]
        indices = cidx[mid]
        reassoc = msim[mid]
        sel = (gtimg == gtimg[indices]) & (gtimg != 255)
        sim = assoc.astype(np.float64) * reassoc.astype(np.float64)
        with np.errstate(divide="ignore"):
            term = np.minimum(-np.log(sim), 100.0)
        total += float((term * sel).sum())
        cnt += float(sel.sum())
    loss = total / max(cnt, 1.0) if cnt > 0 else 0.0
    out = np.float32(loss)
    if _want_profile:
        return out, res
    return out


# revision 31
# speedup vs baseline: 1.8346x; 1.0376x over previous
"""AssociationLoss Trainium2 kernel (8 NeuronCores, SPMD).

Sharding: data-parallel over N=2 images (cores 0-3 -> image 0, cores 4-7 ->
image 1); within an image the [HW, HW] similarity work is sharded row-wise,
1024 rows per core. The spatially-aggregated x2 (spagg) is computed
column-sharded and all-gathered per image group; per-row/column argmax + max
softmax values are produced per shard and the tiny index-chase epilogue
(indices = col_argmax[row_argmax], gt compare, BCE mean over ~16KB/core)
runs on host.

Precision: matmul operands are bf16 (PSUM accumulation fp32). Coherent scale
factors (column norms) are kept fp32 so only incoherent per-element noise
enters the similarity values. Row-standardized softmax scores are invariant to
per-row scaling, so row-side (lhsT) operands are used unnormalized. 1/sqrt is
exp(-0.5*ln(x)) so the whole kernel uses one ACT table set.
"""
import sys, os, json, shutil
sys.path.insert(0, "/opt/trn_rl_repo")
import numpy as np
from concourse import bass, mybir, tile, bacc
from concourse import bass_utils

F32 = mybir.dt.float32
F32R = mybir.dt.float32r
BF16 = mybir.dt.bfloat16
U32 = mybir.dt.uint32
AF = mybir.ActivationFunctionType
ALU = mybir.AluOpType

C = 512
L = 4096
SH = 1024
CCH = 4
NB = L // 512
EPS = 1e-10
N_CORES = 8

MMDT = BF16


_ACT_PATCHED = False


def _ensure_act_json():
    """Steer Bacc's ACT-table-set chooser to natural_log_exp_and_others (it
    contains ln/exp/copy/square — everything this kernel uses) so one
    ACT_TABLE_LOAD suffices instead of per-call set thrashing. Other sets are
    presented as empty (indices preserved for walrus)."""
    # NOTE: steering the chooser by emptying other sets produced wrong ACT
    # results on hardware (runtime table-id mismatch) — disabled.
    return


def build():
    _ensure_act_json()
    nc = bacc.Bacc("TRN2", target_bir_lowering=False, debug=False,
                   num_devices=N_CORES)
    x2_e = nc.declare_dram_parameter("x2", [C, L], MMDT, isOutput=False)
    x2s_e = nc.declare_dram_parameter("x2s", [C, SH], MMDT, isOutput=False)
    x2t_e = nc.declare_dram_parameter("x2t", [L, C], MMDT, isOutput=False)
    x1_e = nc.declare_dram_parameter("x1", [C, L], MMDT, isOutput=False)
    x1s_e = nc.declare_dram_parameter("x1s", [C, SH], MMDT, isOutput=False)
    cst_e = nc.declare_dram_parameter("consts", [129, 128], F32R, isOutput=False)
    out_e = nc.declare_dram_parameter("out", [4, 128, 8], F32, isOutput=True)

    with tile.TileContext(nc) as tc:
        with nc.allow_low_precision(reason="bf16 matmul operands by design"), \
             tc.tile_pool(name="sb", bufs=1) as sb, \
             tc.tile_pool(name="ps", bufs=1, space="PSUM") as ps, \
             tc.tile_pool(name="dr", bufs=1, space="DRAM") as dr:
            _build_body(nc, tc, sb, ps, dr,
                        x2_e, x2s_e, x2t_e, x1_e, x1s_e, cst_e, out_e)
    nc.compile()
    return nc


def _rsqrt_act(nc, sb, dst, src_ap, scale, pref, shape=None):
    """dst = (scale*src)^-0.5 via exp(-0.5*ln(scale*src))."""
    shape = shape or [128, 512]
    lnv = sb.tile(shape, F32, tag="lnscr", bufs=2, name=f"{pref}ln")
    nc.scalar.activation(lnv[:], src_ap, AF.Ln, scale=scale)
    nc.scalar.activation(dst, lnv[:], AF.Exp, scale=-0.5)


def _norm_full_streamed(nc, sb, ps, src_e, out_tiles, ones128b, pref):
    """Column-normalize a [C, L] DRAM tensor into 4x[128, L] bf16 tiles,
    streaming the source twice in [128, 512] chunks. Norm scale stays fp32."""
    npq = [ps.tile([128, SH], F32, tag=f"q{q}", name=f"{pref}npq{q}")
           for q in range(4)]
    nps = [npq[n // 2][:, (n % 2)*512:(n % 2 + 1)*512] for n in range(NB)]
    for cc in range(CCH):
        for n in range(NB):
            ld = sb.tile([128, 512], MMDT, tag="bigload", bufs=4,
                         name=f"{pref}ld{cc}_{n}")
            nc.sync.dma_start(out=ld[:], in_=src_e[cc*128:(cc+1)*128,
                                                   n*512:(n+1)*512])
            sq = sb.tile([128, 512], MMDT, tag="sqchunk", bufs=3,
                         name=f"{pref}sq{cc}_{n}")
            nc.scalar.activation(sq[:], ld[:], AF.Square)
            nc.tensor.matmul(nps[n], lhsT=ones128b[:], rhs=sq[:],
                             start=(cc == 0), stop=(cc == CCH - 1))
    for n in range(NB):
        rn = sb.tile([128, 512], F32, tag="rnorm", bufs=2, name=f"{pref}rn{n}")
        _rsqrt_act(nc, sb, rn[:], nps[n], 1.0, f"{pref}rs{n}")
        for cc in range(CCH):
            ld = sb.tile([128, 512], MMDT, tag="bigload", bufs=4,
                         name=f"{pref}ld2{cc}_{n}")
            nc.sync.dma_start(out=ld[:], in_=src_e[cc*128:(cc+1)*128,
                                                   n*512:(n+1)*512])
            nc.vector.tensor_tensor(out_tiles[cc][:, n*512:(n+1)*512],
                                    ld[:], rn[:], ALU.mult)


def _row_stats(nc, sb, quarters, tag_pref, ib, opack_val, opack_idx):
    """quarters = 4x [128, 1024] PSUM quarters of similarity rows.
    Standardized softmax stats per row; argmax + max softmax value to opack."""
    s12 = sb.tile([128, L], F32, tag="srow", bufs=3, name=f"{tag_pref}s{ib}")
    bno = sb.tile([128, 6 * NB], F32, tag="bno", bufs=2, name=f"{tag_pref}bno{ib}")
    for q in range(4):
        nc.scalar.activation(s12[:, q*SH:(q+1)*SH], quarters[q][:], AF.Copy)
        for h in range(2):
            n = q*2 + h
            nc.vector.bn_stats(bno[:, n*6:(n+1)*6],
                               quarters[q][:, h*512:(h+1)*512])
    bna = sb.tile([128, 2], F32, tag="bna", bufs=2, name=f"{tag_pref}bna{ib}")
    nc.vector.bn_aggr(bna[:], bno[:])
    m8 = sb.tile([128, 8], F32, tag="m8", bufs=2, name=f"{tag_pref}m8{ib}")
    nc.vector.max(m8[:], s12[:])
    mi8 = sb.tile([128, 8], U32, tag="mi8", bufs=2, name=f"{tag_pref}mi8{ib}")
    nc.vector.max_index(mi8[:], m8[:], s12[:])
    nc.vector.tensor_copy(opack_idx[:, ib:ib+1], mi8[:, 0:1])

    def tiny(nm):
        return sb.tile([128, 1], F32, tag="tiny", bufs=16, name=f"{tag_pref}{nm}{ib}")
    rinv = tiny("rinv")
    _rsqrt_act(nc, sb, rinv[:], bna[:, 1:2], L / (L - 1.0), f"{tag_pref}ri{ib}",
               shape=[128, 1])
    mr = tiny("mr")
    nc.vector.tensor_tensor(mr[:], bna[:, 0:1], rinv[:], ALU.mult)
    nb_ = tiny("nb")
    nc.vector.tensor_scalar_mul(nb_[:], mr[:], -1.0)
    rse = tiny("rse")
    trash = sb.tile([128, L], BF16, tag="trash", bufs=2, name=f"{tag_pref}t2{ib}")
    nc.scalar.activation(trash[:], s12[:], AF.Exp, scale=rinv[:], bias=nb_[:],
                         accum_out=rse[:])
    t3 = tiny("t3")
    nc.vector.tensor_tensor(t3[:], m8[:, 0:1], rinv[:], ALU.mult)
    t4 = tiny("t4")
    nc.vector.tensor_tensor(t4[:], t3[:], nb_[:], ALU.add)
    em = tiny("em")
    nc.scalar.activation(em[:], t4[:], AF.Exp)
    rr2 = tiny("rr2")
    nc.vector.reciprocal(rr2[:], rse[:])
    nc.vector.tensor_tensor(opack_val[:, ib:ib+1], em[:], rr2[:], ALU.mult)


def _build_body(nc, tc, sb, ps, dr, x2_e, x2s_e, x2t_e, x1_e, x1s_e, cst_e,
                out_e):
    # ---- constants: f32r ones/identity for the stats transpose, bf16 ones
    # for partition-sum matmuls ----
    ones1 = sb.tile([1, 128], F32R, name="ones1")
    nc.sync.dma_start(out=ones1[:], in_=cst_e[0:1, :])
    id128 = sb.tile([128, 128], F32R, name="id128")
    nc.sync.dma_start(out=id128[:], in_=cst_e[1:129, :])
    ones1b = sb.tile([1, 128], BF16, name="ones1b")
    nc.vector.tensor_copy(ones1b[:], ones1[:])
    ones128b = sb.tile([128, 128], BF16, name="ones128b")
    nc.gpsimd.partition_broadcast(ones128b[:], ones1b[:])

    opack = [sb.tile([128, 8], F32, tag=f"op{k}", name=f"opack{k}") for k in range(4)]

    # ---- phase A: normalize x2 columns into resident x2n (bf16) ----
    x2sh = []
    for cc in range(CCH):
        t = sb.tile([128, SH], MMDT, tag=f"shA{cc}", bufs=1, name=f"x2sh{cc}")
        nc.sync.dma_start(out=t[:], in_=x2s_e[cc*128:(cc+1)*128, :])
        x2sh.append(t)
    x2n = [sb.tile([128, L], MMDT, tag=f"bigB{cc}", bufs=1, name=f"x2n{cc}")
           for cc in range(CCH)]
    _norm_full_streamed(nc, sb, ps, x2_e, x2n, ones128b, "A")
    x1n = [sb.tile([128, L], MMDT, tag=f"bigC{cc}", bufs=1, name=f"x1n{cc}")
           for cc in range(CCH)]
    _norm_full_streamed(nc, sb, ps, x1_e, x1n, ones128b, "D")
    x1sh = []
    for cc in range(CCH):
        t = sb.tile([128, SH], MMDT, tag=f"shA{cc}", bufs=1, name=f"x1sh{cc}")
        nc.sync.dma_start(out=t[:], in_=x1s_e[cc*128:(cc+1)*128, :])
        x1sh.append(t)

    # ---- phase B: sim_self row stats (raw lhsT; row-scale invariant) ----
    spack = sb.tile([128, 16], F32R, name="spack")
    for ib in range(8):
        qs = [ps.tile([128, SH], F32, tag=f"q{q}", name=f"B_q{q}_{ib}")
              for q in range(4)]
        for q in range(4):
            for cc in range(CCH):
                for h in range(2):
                    n = q*2 + h
                    nc.tensor.matmul(qs[q][:, h*512:(h+1)*512],
                                     lhsT=x2sh[cc][:, ib*128:(ib+1)*128],
                                     rhs=x2n[cc][:, n*512:(n+1)*512],
                                     start=(cc == 0), stop=(cc == CCH - 1))
        bno = sb.tile([128, 6 * NB], F32, tag="bno", bufs=2, name=f"Bbno{ib}")
        for q in range(4):
            for h in range(2):
                n = q*2 + h
                nc.vector.bn_stats(bno[:, n*6:(n+1)*6],
                                   qs[q][:, h*512:(h+1)*512])
        bna = sb.tile([128, 2], F32, tag="bna", bufs=2, name=f"Bbna{ib}")
        nc.vector.bn_aggr(bna[:], bno[:])
        rinv = sb.tile([128, 1], F32, tag="tiny", bufs=16, name=f"Brinv{ib}")
        _rsqrt_act(nc, sb, rinv[:], bna[:, 1:2], L / (L - 1.0), f"Bri{ib}",
                   shape=[128, 1])
        nc.vector.tensor_copy(spack[:, ib:ib+1], rinv[:])
        mr = sb.tile([128, 1], F32, tag="tiny", bufs=16, name=f"Bmr{ib}")
        nc.vector.tensor_tensor(mr[:], bna[:, 0:1], rinv[:], ALU.mult)
        nc.vector.tensor_scalar_mul(spack[:, 8+ib:9+ib], mr[:], -1.0)
    stT = ps.tile([16, 128], F32R, tag="q0", name="stT")
    nc.tensor.transpose(stT[:], spack[:], id128[:])
    st = sb.tile([16, 128], F32R, name="st")
    nc.vector.tensor_copy(st[:], stT[:])
    # move rinv/bias rows to partition 0 as [1, 1024] rows (DRAM bounce)
    b_dram = dr.tile([1, SH], F32R, name="b_dram")
    nc.sync.dma_start(out=b_dram[:, :].rearrange("o (a b) -> (o a) b", a=8),
                      in_=st[8:16, :])
    b_rowf = sb.tile([1, SH], F32R, name="b_rowf")
    nc.sync.dma_start(out=b_rowf[:], in_=b_dram[:, :])
    b_row = sb.tile([1, SH], BF16, name="b_row")
    nc.vector.tensor_copy(b_row[:], b_rowf[:])
    r_dram = dr.tile([1, SH], F32R, name="r_dram")
    nc.sync.dma_start(out=r_dram[:, :].rearrange("o (a b) -> (o a) b", a=8),
                      in_=st[0:8, :])
    r_rowf = sb.tile([1, SH], F32R, name="r_rowf")
    nc.sync.dma_start(out=r_rowf[:], in_=r_dram[:, :])

    # ---- phase C: sim_self exp + agg + spagg combine + renormalize ----
    cc_in = dr.tile([C, SH], MMDT, name="cc_in")
    xnn = {}
    for ic in range(2):
        rbc = sb.tile([128, 512], F32R, tag="rbc", bufs=1, name=f"rbc{ic}")
        nc.gpsimd.partition_broadcast(rbc[:], r_rowf[0:1, ic*512:(ic+1)*512])
        ys = []
        for cc in range(CCH):
            y = sb.tile([128, 512], MMDT, tag=f"y{cc}", bufs=1, name=f"y{cc}_{ic}")
            nc.vector.tensor_tensor(y[:], x2sh[cc][:, ic*512:(ic+1)*512], rbc[:],
                                    ALU.mult)
            ys.append(y)
        q0 = ps.tile([128, SH], F32, tag="q0", name=f"C_q0_{ic}")
        q1 = ps.tile([128, SH], F32, tag="q1", name=f"C_q1_{ic}")
        q2 = ps.tile([128, SH], F32, tag="q2", name=f"C_q2_{ic}")
        q3 = ps.tile([128, SH], F32, tag="q3", name=f"C_q3_{ic}")
        # bank map: q0 = [zt0, zt1]; q1/q2 = agg; q3 = [rse, np2]
        pa = q0
        agg_ps = [q1[:, 0:512], q1[:, 512:1024], q2[:, 0:512], q2[:, 512:1024]]
        rse_ps = q3[0:1, 0:512]
        for jb in range(32):
            zt = pa[:, (jb % 2)*512:(jb % 2 + 1)*512]
            for cc in range(CCH):
                nc.tensor.matmul(zt, lhsT=x2n[cc][:, jb*128:(jb+1)*128],
                                 rhs=ys[cc][:], start=(cc == 0), stop=False)
            nc.tensor.matmul(zt, lhsT=ones1b[:],
                             rhs=b_row[0:1, ic*512:(ic+1)*512],
                             start=False, stop=True)
            ez = sb.tile([128, 512], MMDT, tag="ez", bufs=3, name=f"ez{ic}_{jb}")
            nc.scalar.activation(ez[:], zt, AF.Exp)
            xt = sb.tile([128, C], MMDT, tag="xt", bufs=3, name=f"xt{ic}_{jb}")
            nc.sync.dma_start(out=xt[:], in_=x2t_e[jb*128:(jb+1)*128, :])
            for cc in range(CCH):
                nc.tensor.matmul(agg_ps[cc], lhsT=xt[:, cc*128:(cc+1)*128],
                                 rhs=ez[:], start=(jb == 0), stop=(jb == 31))
            nc.tensor.matmul(rse_ps, lhsT=ones128b[:, 0:1], rhs=ez[:],
                             start=(jb == 0), stop=(jb == 31))
        rrse = sb.tile([1, 512], F32, tag="rrse", bufs=1, name=f"rrse{ic}")
        nc.vector.reciprocal(rrse[:], rse_ps)
        rrb = sb.tile([128, 512], F32, tag="rrb", bufs=1, name=f"rrb{ic}")
        nc.gpsimd.partition_broadcast(rrb[:], rrse[:])
        nps2 = q3[:, 512:1024]
        xss_l = []
        for cc in range(CCH):
            tmp = sb.tile([128, 512], F32, tag="tmpc", bufs=1, name=f"tmp{cc}_{ic}")
            nc.vector.tensor_tensor(tmp[:], agg_ps[cc], rrb[:], ALU.mult)
            xss = sb.tile([128, 512], F32, tag="xss", bufs=4, name=f"xss{cc}_{ic}")
            nc.vector.tensor_tensor(xss[:], tmp[:], x2sh[cc][:, ic*512:(ic+1)*512],
                                    ALU.add)
            xss_l.append(xss)
            sq2 = sb.tile([128, 512], MMDT, tag="sqchunk", bufs=3,
                          name=f"sq2{cc}_{ic}")
            nc.scalar.activation(sq2[:], xss[:], AF.Square)
            nc.tensor.matmul(nps2, lhsT=ones128b[:], rhs=sq2[:],
                             start=(cc == 0), stop=(cc == CCH - 1))
        rnb2 = sb.tile([128, 512], F32, tag="rnb2", bufs=1, name=f"rnb2{ic}")
        _rsqrt_act(nc, sb, rnb2[:], nps2, 1.0, f"rnb2s{ic}")
        for cc in range(CCH):
            xo = sb.tile([128, 512], MMDT, tag="xnn", bufs=8, name=f"xnn{cc}_{ic}")
            nc.vector.tensor_tensor(xo[:], xss_l[cc][:], rnb2[:], ALU.mult)
            xnn[(cc, ic)] = xo
            nc.sync.dma_start(out=cc_in[cc*128:(cc+1)*128, ic*512:(ic+1)*512],
                              in_=xo[:])

    # ---- all-gather normalized x2_new within each image group ----
    cc_out = dr.tile([4, C, SH], MMDT, name="cc_out")
    nc.gpsimd.collective_compute(
        "AllGather", ALU.bypass,
        replica_groups=[[0, 1, 2, 3], [4, 5, 6, 7]],
        ins=[cc_in[:].opt()], outs=[cc_out[:].opt()])

    # ---- phase F: raw21 columns (fully local; overlaps the all-gather) ----
    for jb in range(8):
        qs = [ps.tile([128, SH], F32, tag=f"q{q}", name=f"F_q{q}_{jb}")
              for q in range(4)]
        for q in range(4):
            for cc in range(CCH):
                for h in range(2):
                    n = q*2 + h
                    nc.tensor.matmul(qs[q][:, h*512:(h+1)*512],
                                     lhsT=xnn[(cc, jb // 4)][:, (jb % 4)*128:(jb % 4 + 1)*128],
                                     rhs=x1n[cc][:, n*512:(n+1)*512],
                                     start=(cc == 0), stop=(cc == CCH - 1))
        _row_stats(nc, sb, qs, "F", jb, opack_val=opack[3], opack_idx=opack[2])

    # ---- read back gathered x2_new (reuses bigB after x1n's last use) ----
    xg = []
    for cc in range(CCH):
        t = sb.tile([128, L], MMDT, tag=f"bigB{cc}", bufs=1, name=f"xg{cc}")
        for rr in range(4):
            nc.sync.dma_start(out=t[:, rr*SH:(rr+1)*SH],
                              in_=cc_out[rr, cc*128:(cc+1)*128, :])
        xg.append(t)

    # ---- phase E: raw12 rows ----
    for ib in range(8):
        qs = [ps.tile([128, SH], F32, tag=f"q{q}", name=f"E_q{q}_{ib}")
              for q in range(4)]
        for q in range(4):
            for cc in range(CCH):
                for h in range(2):
                    n = q*2 + h
                    nc.tensor.matmul(qs[q][:, h*512:(h+1)*512],
                                     lhsT=x1sh[cc][:, ib*128:(ib+1)*128],
                                     rhs=xg[cc][:, n*512:(n+1)*512],
                                     start=(cc == 0), stop=(cc == CCH - 1))
        _row_stats(nc, sb, qs, "E", ib, opack_val=opack[1], opack_idx=opack[0])

    # ---- outputs ----
    for k in range(4):
        nc.sync.dma_start(out=out_e[k, :, :], in_=opack[k][:])


_NC_CACHE = None


def _get_nc():
    global _NC_CACHE
    if _NC_CACHE is None:
        _NC_CACHE = build()
    return _NC_CACHE


def _consts():
    c = np.zeros((129, 128), np.float32)
    c[0, :] = 1.0
    c[1:129, :] = np.eye(128, dtype=np.float32)
    return c


def _unpack(v):
    # v: [128, 8] packed [p, b] -> flat vec[b*128 + p]
    return v.T.reshape(-1)


def kernel(x1, x2, gt1, _want_profile=False):
    N = x1.shape[0]
    assert N == 2 and x1.shape == (2, 512, 64, 64)
    bf = mybir.dt.np(BF16)
    x1m = x1.reshape(N, C, L).astype(np.float32)
    x2m = x2.reshape(N, C, L).astype(np.float32)
    gtv = gt1.reshape(N, L)

    in_maps = []
    for core in range(N_CORES):
        g, r = core // 4, core % 4
        sl = slice(r * SH, (r + 1) * SH)
        in_maps.append({
            "x2": np.ascontiguousarray(x2m[g]).astype(bf),
            "x2s": np.ascontiguousarray(x2m[g][:, sl]).astype(bf),
            "x2t": np.ascontiguousarray(x2m[g].T).astype(bf),
            "x1": np.ascontiguousarray(x1m[g]).astype(bf),
            "x1s": np.ascontiguousarray(x1m[g][:, sl]).astype(bf),
            "consts": _consts(),
        })
    nc = _get_nc()
    res = bass_utils.run_bass_kernel_spmd(
        nc, in_maps, core_ids=list(range(N_CORES)),
        trace=_want_profile)

    total = 0.0
    cnt = 0.0
    for g in range(N):
        packs = [res.results[g * 4 + r]["out"] for r in range(4)]
        mid = np.concatenate([_unpack(p[0]) for p in packs]).astype(np.int64)
        assoc = np.concatenate([_unpack(p[1]) for p in packs])
        cidx = np.concatenate([_unpack(p[2]) for p in packs]).astype(np.int64)
        msim = np.concatenate([_unpack(p[3]) for p in packs])
        gtimg = gtv[g]
        indices = cidx[mid]
        reassoc = msim[mid]
        sel = (gtimg == gtimg[indices]) & (gtimg != 255)
        sim = assoc.astype(np.float64) * reassoc.astype(np.float64)
        with np.errstate(divide="ignore"):
            term = np.minimum(-np.log(sim), 100.0)
        total += float((term * sel).sum())
        cnt += float(sel.sum())
    loss = total / max(cnt, 1.0) if cnt > 0 else 0.0
    out = np.float32(loss)
    if _want_profile:
        return out, res
    return out
